# revision 1
# baseline (speedup 1.0000x reference)
"""Self-contained Trainium2 Bass kernel for nn_MultiHeadMPSRecurrence.

Reference computation (B=2, T=4096, D_MODEL=1024, D_HIDDEN=2048, K=4):
    ih    = causal_depthwise_conv(x @ W_ih + b, conv_w, conv_b)
    gate  = sigmoid(x @ W_gate + b)
    a     = sigmoid(x @ W_decay + b)
    z     = silu(x @ W_z + b)
    alpha = (1-gate)*a ; beta = gate*ih
    h     = scan(h_t = alpha_t*h_{t-1} + beta_t)
    out   = ((z * layernorm(h)) @ W_out + b) @ out_w + b

Sharding: 8 cores = 2 batches x 4 time-chunks of 1024 steps.  Each core runs
the full pipeline on its chunk; the sequential scan is chunk-linked through
AllGathers of per-chunk (prod(alpha), last-local-state) vectors followed by an
on-device prefix combine, then each core re-runs its scan with the true
incoming state (native tensor_tensor_scan ISA op, fp32 internal state).  The
exchange is split into two collectives so most of the post-exchange work
overlaps the tail of the main matmul phase.

On-chip layout: channels on partitions (16 tiles of 128ch), time on free dim.
Matmuls run in bf16 with fp32 PSUM accumulation; elementwise/scan math in
fp32.  Host pre-fuses W_out @ out_w into one [2048,1024] matrix.
"""

import functools
from contextlib import ExitStack

import ml_dtypes
import numpy as np

import concourse.tile as tile
from concourse import bacc, mybir
from concourse.bass_utils import run_bass_kernel_spmd

BF16 = mybir.dt.bfloat16
F32 = mybir.dt.float32
AF = mybir.ActivationFunctionType
OP = mybir.AluOpType
AX = mybir.AxisListType

B, T, D, H = 2, 4096, 1024, 2048
NCORES = 8
KCHUNKS = 4          # time chunks per batch
TC = T // KCHUNKS    # 1024 timesteps per core
HALO = 3             # conv taps reaching back in time (K-1)
TCX = TC + HALO      # 1027
NCT = H // 128       # 16 channel tiles
NDT = D // 128       # 8 d_model tiles
LN_EPS = 1e-5
SPLIT = 12           # channel tiles covered by the first boundary exchange


def _build_program(has_out_bias: bool, has_ln_b: bool, sim_no_cc: bool = False,
                   repeat: int = 1):
    nc = bacc.Bacc("TRN2", target_bir_lowering=False, debug=False,
                   enable_asserts=True, num_devices=NCORES)

    # ---------------- I/O ----------------
    xT = nc.dram_tensor("xT", [D, TCX], BF16, kind="ExternalInput").ap()
    w4 = nc.dram_tensor("w4", [D, 4 * H], BF16, kind="ExternalInput").ap()
    wcomb = nc.dram_tensor("wcomb", [H, D], BF16, kind="ExternalInput").ap()
    # per-channel vectors in [partition, ch_tile] layout
    def cvec(name, n=NCT):
        return nc.dram_tensor(name, [128, n], F32, kind="ExternalInput").ap()
    bias_ih = cvec("bias_ih")
    bias_halo = cvec("bias_halo")
    bias_gneg = cvec("bias_gneg")
    bias_dec = cvec("bias_dec")
    bias_z = cvec("bias_z")
    conv_b_d = cvec("conv_b")
    ln_g_d = cvec("ln_g")
    convw_d = cvec("convw", NCT * 4)
    onehot_d = cvec("onehot", NCORES)
    if has_out_bias:
        out_bias_d = nc.dram_tensor("out_bias", [128, D], BF16,
                                    kind="ExternalInput").ap()
    if has_ln_b:
        wcomb_b = nc.dram_tensor("wcomb_b", [H, D], BF16,
                                 kind="ExternalInput").ap()
    out = nc.dram_tensor("out", [TC, D], F32, kind="ExternalOutput").ap()

    with tile.TileContext(nc) as tc, ExitStack() as ctx:
        dram = ctx.enter_context(tc.tile_pool(name="dram", bufs=1, space="DRAM"))
        alpha_d = dram.tile([H, TC], BF16)
        beta_d = dram.tile([H, TC], BF16)
        n1, n2 = SPLIT, NCT - SPLIT
        rstd_d = dram.tile([2, TC], F32)

        # ---------------- constants / small resident ----------------
        consts = ctx.enter_context(tc.tile_pool(name="consts", bufs=1))

        def load_const(name, ap_in, n):
            t = consts.tile([128, n], F32, tag=name)
            nc.sync.dma_start(t[:], ap_in[:, :])
            return t
        bih_s = load_const("bih", bias_ih, NCT)
        bhalo_s = load_const("bhalo", bias_halo, NCT)
        bgn_s = load_const("bgn", bias_gneg, NCT)
        bdec_s = load_const("bdec", bias_dec, NCT)
        bz_s = load_const("bz", bias_z, NCT)
        cb_s = load_const("cb", conv_b_d, NCT)
        lng_s = load_const("lng", ln_g_d, NCT)
        cw_s = load_const("cw", convw_d, NCT * 4)
        oh_s = load_const("oh", onehot_d, NCORES)
        if has_out_bias:
            obias_s = consts.tile([128, D], BF16, tag="obias")
            nc.sync.dma_start(obias_s[:], out_bias_d[:, :])
        ones_stat = consts.tile([128, 1], BF16)
        nc.vector.memset(ones_stat[:], 1.0)
        ones_bc = consts.tile([1, 128], F32)
        nc.vector.memset(ones_bc[:], 1.0)

        # ---------------- big resident tiles ----------------
        res = ctx.enter_context(tc.tile_pool(name="res", bufs=1))
        z_s = res.tile([128, NCT * TC], BF16)
        h_s = res.tile([128, NCT * TC], BF16)
        wcomb_s = res.tile([128, NCT * D], BF16)
        rows = ctx.enter_context(tc.tile_pool(name="rows", bufs=1))
        Pbuf = res.tile([128, NCT], F32)
        Lbuf = res.tile([128, NCT], F32)
        h_in = res.tile([128, NCT], F32)
        Pall = res.tile([128, NCORES * NCT], F32)
        Lall = res.tile([128, NCORES * NCT], F32)
        Hp = res.tile([128, NCORES * NCT], F32)
        rstd_col = res.tile([128, NDT], F32)
        cumA_s = res.tile([128, (NCT - SPLIT) * TC], BF16)
        mu_sb = rows.tile([128, TC], F32)
        mu_row = rows.tile([1, TC], F32)
        msq_row = rows.tile([1, TC], F32)
        mu = mu_row[:, :]
        msq = msq_row[:, :]
        cpack = rows.tile([128, 4 * NDT + 1], F32)
        mu_col = cpack[:, 0:NDT]
        msq_col = cpack[:, NDT:2 * NDT]
        var_col = cpack[:, 2 * NDT:3 * NDT]
        sd_col = cpack[:, 3 * NDT:4 * NDT]
        eps_t = cpack[:, 4 * NDT:4 * NDT + 1]

        def emit_body():
            cc1_in = dram.tile([1, 2 * n1 * 128], F32, tag="cc1i")
            cc1_out = dram.tile([NCORES, 2 * n1 * 128], F32,
                                addr_space="Shared", tag="cc1o")
            cc2_in = dram.tile([1, 2 * n2 * 128], F32, tag="cc2i")
            cc2_out = dram.tile([NCORES, 2 * n2 * 128], F32,
                                addr_space="Shared", tag="cc2o")
            def comm_half(lo, hi, cci, cco):
                """AllGather (P,L) for channel tiles [lo,hi); prefix-combine and
                select this core's incoming state into h_in[:, lo:hi]."""
                n = hi - lo
                nc.sync.dma_start(
                    cci[0:1, 0:n * 128].rearrange("o (ct p) -> p (o ct)", p=128),
                    Pbuf[:, lo:hi])
                nc.sync.dma_start(
                    cci[0:1, n * 128:2 * n * 128]
                    .rearrange("o (ct p) -> p (o ct)", p=128),
                    Lbuf[:, lo:hi])
                if sim_no_cc:
                    # TimelineSim can't model collectives; local DMA stand-in.
                    nc.sync.dma_start(cco[0:1, :], cci[:, :])
                else:
                    nc.gpsimd.collective_compute(
                        "AllGather", OP.bypass,
                        replica_groups=[list(range(NCORES))],
                        ins=[cci.opt()], outs=[cco.opt()])
                for j in range(NCORES):
                    nc.sync.dma_start(
                        Pall[:, j * NCT + lo: j * NCT + hi],
                        cco[j:j + 1, 0:n * 128]
                        .rearrange("o (ct p) -> p (o ct)", p=128))
                    nc.sync.dma_start(
                        Lall[:, j * NCT + lo: j * NCT + hi],
                        cco[j:j + 1, n * 128:2 * n * 128]
                        .rearrange("o (ct p) -> p (o ct)", p=128))
                for j in range(NCORES):
                    sj = slice(j * NCT + lo, j * NCT + hi)
                    sjm = slice((j - 1) * NCT + lo, (j - 1) * NCT + hi)
                    if j % KCHUNKS == 0:
                        nc.vector.tensor_copy(Hp[:, sj], Lall[:, sj])
                    else:
                        nc.vector.tensor_tensor(Hp[:, sj], Pall[:, sj],
                                                Hp[:, sjm], OP.mult)
                        nc.vector.tensor_tensor(Hp[:, sj], Hp[:, sj],
                                                Lall[:, sj], OP.add)
                nc.vector.memset(h_in[:, lo:hi], 0.0)
                for j in range(NCORES):
                    sj = slice(j * NCT + lo, j * NCT + hi)
                    nc.vector.scalar_tensor_tensor(
                        h_in[:, lo:hi], Hp[:, sj], oh_s[:, j:j + 1],
                        h_in[:, lo:hi], OP.mult, OP.add)

            # distribute first-half true-scans over the tail of phase 1
            p3_sched = {SPLIT + i: [] for i in range(NCT - SPLIT)}
            for i in range(SPLIT):
                p3_sched[SPLIT + min(i * (NCT - SPLIT) // SPLIT,
                                     NCT - SPLIT - 1)].append(i)

            with tc.tile_pool(name="p3ps", bufs=1, space="PSUM") as spool, \
                 tc.tile_pool(name="p3b", bufs=2) as p3pool:
                psum_sh = spool.tile([1, TC], F32)
                psum_sq = spool.tile([1, TC], F32)

                def phase3_ct(ct):
                    al3 = p3pool.tile([128, TC], BF16, tag="al3")
                    nc.sync.dma_start(al3[:], alpha_d[ct * 128:(ct + 1) * 128, :])
                    be3 = p3pool.tile([128, TC], BF16, tag="be3")
                    nc.sync.dma_start(be3[:], beta_d[ct * 128:(ct + 1) * 128, :])
                    hsl = h_s[:, ct * TC:(ct + 1) * TC]
                    nc.vector.tensor_tensor_scan(hsl, al3[:], be3[:],
                                                 h_in[:, ct:ct + 1],
                                                 OP.mult, OP.add)
                    for half in range(2):
                        nc.tensor.matmul(
                            psum_sh[0:1, half * 512:(half + 1) * 512],
                            ones_stat[:, 0:1],
                            h_s[:, ct * TC + half * 512: ct * TC + (half + 1) * 512],
                            start=(ct == 0), stop=(ct == NCT - 1))
                    h2 = p3pool.tile([128, TC], BF16, tag="h2")
                    nc.gpsimd.tensor_tensor(h2[:], hsl, hsl, OP.mult)
                    for half in range(2):
                        nc.tensor.matmul(
                            psum_sq[0:1, half * 512:(half + 1) * 512],
                            ones_stat[:, 0:1],
                            h2[:, half * 512:(half + 1) * 512],
                            start=(ct == 0), stop=(ct == NCT - 1))

                # ============ phase 1: matmuls + gates + local scans ==========
                with tc.tile_pool(name="p1xt", bufs=1) as xtpool, \
                     tc.tile_pool(name="p1w", bufs=2) as wpool, \
                     tc.tile_pool(name="p1ps", bufs=4, space="PSUM") as ppool, \
                     tc.tile_pool(name="p1f", bufs=2) as fpool, \
                     tc.tile_pool(name="p1b",
                              bufs=1 if (has_out_bias or has_ln_b) else 2) \
                 as bpool:
                    xT_s = xtpool.tile([128, NDT * TCX], BF16)

                    def load_w(ct):
                        w_t = wpool.tile([128, 4 * NDT * 128], BF16, tag="w_t")
                        for m in range(4):
                            nc.sync.dma_start(
                                w_t[:, m * NDT * 128:(m + 1) * NDT * 128]
                                .rearrange("p (dt c) -> p dt c", c=128),
                                w4[:, m * H + ct * 128: m * H + (ct + 1) * 128]
                                .rearrange("(dt p) c -> p dt c", p=128),
                            )
                        return w_t

                    # first weight tile ahead of the bulk xT load so PE can start
                    w_next = load_w(0)
                    for dt in range(NDT):
                        nc.gpsimd.dma_start(
                            xT_s[:, dt * TCX:(dt + 1) * TCX],
                            xT[dt * 128:(dt + 1) * 128, :].rearrange(
                                "(o p) t -> p (o t)", p=128))

                    for ct in range(NCT):
                        w_t = w_next
                        if ct + 1 < NCT:
                            w_next = load_w(ct + 1)

                        def mmgroup(m, lo, n):
                            ps = ppool.tile([128, 512], F32, tag="ps")
                            for dt in range(NDT):
                                nc.tensor.matmul(
                                    ps[:, 0:n],
                                    w_t[:, (m * NDT + dt) * 128:
                                        (m * NDT + dt + 1) * 128],
                                    xT_s[:, dt * TCX + lo: dt * TCX + lo + n],
                                    start=(dt == 0), stop=(dt == NDT - 1),
                                )
                            return ps

                        # m=0: ih over all 1027 halo columns
                        ih_pre = fpool.tile([128, TCX], F32, tag="ihpre")
                        for (lo, n, bias) in ((0, HALO, bhalo_s),
                                              (HALO, 512, bih_s),
                                              (HALO + 512, 512, bih_s)):
                            ps = mmgroup(0, lo, n)
                            nc.scalar.activation(ih_pre[:, lo:lo + n], ps[:, 0:n],
                                                 AF.Identity,
                                                 bias=bias[:, ct:ct + 1])
                        # m=1: gp = sigmoid(-(x@Wg + bg)) = 1-gate
                        gp = fpool.tile([128, TC], F32, tag="gp")
                        for half in range(2):
                            ps = mmgroup(1, HALO + half * 512, 512)
                            nc.scalar.activation(
                                gp[:, half * 512:(half + 1) * 512], ps[:, :],
                                AF.Sigmoid, bias=bgn_s[:, ct:ct + 1], scale=-1.0)
                        # m=2: a = sigmoid(x@Wd + bd)
                        a_t = fpool.tile([128, TC], F32, tag="a")
                        for half in range(2):
                            ps = mmgroup(2, HALO + half * 512, 512)
                            nc.scalar.activation(
                                a_t[:, half * 512:(half + 1) * 512], ps[:, :],
                                AF.Sigmoid, bias=bdec_s[:, ct:ct + 1])
                        # m=3: z = silu(x@Wz + bz), straight into resident bf16
                        for half in range(2):
                            ps = mmgroup(3, HALO + half * 512, 512)
                            nc.scalar.activation(
                                z_s[:, ct * TC + half * 512:
                                    ct * TC + (half + 1) * 512],
                                ps[:, :], AF.Silu, bias=bz_s[:, ct:ct + 1])

                        # causal depthwise conv (4 taps, halo in ih_pre)
                        ihc = fpool.tile([128, TC], F32, tag="ihc")
                        nc.scalar.activation(ihc[:], ih_pre[:, 3:3 + TC],
                                             AF.Identity,
                                             bias=cb_s[:, ct:ct + 1],
                                             scale=cw_s[:, ct * 4 + 3: ct * 4 + 4])
                        for j in (2, 1, 0):
                            nc.vector.scalar_tensor_tensor(
                                ihc[:], ih_pre[:, j:j + TC],
                                cw_s[:, ct * 4 + j: ct * 4 + j + 1],
                                ihc[:], OP.mult, OP.add)

                        # alpha = gp*a ; reuse a tile: gate = 1-gp ; beta = gate*ihc
                        alpha_t = bpool.tile([128, TC], BF16, tag="alpha")
                        aeng = nc.gpsimd if ct >= SPLIT else nc.vector
                        aeng.tensor_tensor(alpha_t[:], gp[:], a_t[:], OP.mult)
                        nc.scalar.activation(a_t[:], gp[:], AF.Copy,
                                             bias=1.0, scale=-1.0)
                        beta_t = bpool.tile([128, TC], BF16, tag="beta")
                        nc.gpsimd.tensor_tensor(beta_t[:], a_t[:], ihc[:], OP.mult)
                        if ct < SPLIT:
                            nc.sync.dma_start(alpha_d[ct * 128:(ct + 1) * 128, :],
                                              alpha_t[:])
                            nc.sync.dma_start(beta_d[ct * 128:(ct + 1) * 128, :],
                                              beta_t[:])

                        # local scan (init 0) -> L ; running product -> P
                        if ct < SPLIT:
                            h0_t = bpool.tile([128, TC], BF16, tag="h0")
                            nc.vector.tensor_tensor_scan(h0_t[:], alpha_t[:],
                                                         beta_t[:], 0.0,
                                                         OP.mult, OP.add)
                            nc.vector.tensor_copy(Lbuf[:, ct:ct + 1],
                                                  h0_t[:, TC - 1:TC])
                            nc.vector.tensor_reduce(Pbuf[:, ct:ct + 1], alpha_t[:],
                                                    AX.X, OP.mult)
                        else:
                            # tail group: local scan lands in h_s; cumulative
                            # product of alpha kept for the post-exchange fixup
                            # h = h0 + cumA * h_in (no re-scan needed).
                            hsl = h_s[:, ct * TC:(ct + 1) * TC]
                            nc.vector.tensor_tensor_scan(hsl, alpha_t[:],
                                                         beta_t[:], 0.0,
                                                         OP.mult, OP.add)
                            casl = cumA_s[:, (ct - SPLIT) * TC:
                                          (ct - SPLIT + 1) * TC]
                            nc.vector.tensor_tensor_scan(casl, alpha_t[:],
                                                         alpha_t[:], 1.0,
                                                         OP.mult, OP.bypass)
                            nc.vector.tensor_copy(Lbuf[:, ct:ct + 1],
                                                  hsl[:, TC - 1:TC])
                            nc.vector.tensor_copy(Pbuf[:, ct:ct + 1],
                                                  casl[:, TC - 1:TC])

                        if ct == SPLIT - 1:
                            comm_half(0, SPLIT, cc1_in, cc1_out)
                        for c3 in p3_sched.get(ct, []):
                            phase3_ct(c3)

                # ============ second boundary exchange + fixups ===============
                comm_half(SPLIT, NCT, cc2_in, cc2_out)
                for ct in range(SPLIT, NCT):
                    hsl = h_s[:, ct * TC:(ct + 1) * TC]
                    casl = cumA_s[:, (ct - SPLIT) * TC:(ct - SPLIT + 1) * TC]
                    nc.vector.scalar_tensor_tensor(
                        hsl, casl, h_in[:, ct:ct + 1], hsl, OP.mult, OP.add)
                    for half in range(2):
                        nc.tensor.matmul(
                            psum_sh[0:1, half * 512:(half + 1) * 512],
                            ones_stat[:, 0:1],
                            h_s[:, ct * TC + half * 512: ct * TC + (half + 1) * 512],
                            start=(ct == 0), stop=(ct == NCT - 1))
                    h2f = p3pool.tile([128, TC], BF16, tag="h2")
                    nc.scalar.activation(h2f[:], hsl, AF.Square)
                    for half in range(2):
                        nc.tensor.matmul(
                            psum_sq[0:1, half * 512:(half + 1) * 512],
                            ones_stat[:, 0:1],
                            h2f[:, half * 512:(half + 1) * 512],
                            start=(ct == 0), stop=(ct == NCT - 1))

                # ============ LN parameters ===================================
                nc.scalar.activation(mu, psum_sh[:], AF.Copy, scale=1.0 / H)
                nc.scalar.activation(msq, psum_sq[:], AF.Copy, scale=1.0 / H)

            # broadcast mu first: it gates the apply phase; the rstd
            # column chain only gates the final evictions
            with tc.tile_pool(name="p4ps", bufs=1, space="PSUM") as bpool4:
                mu_b = bpool4.tile([128, TC], F32)
                for half in range(2):
                    nc.tensor.matmul(mu_b[:, half * 512:(half + 1) * 512],
                                     ones_bc[0:1, :],
                                     mu[:, half * 512:(half + 1) * 512])
                nc.scalar.activation(mu_sb[:], mu_b[:], AF.Copy)
            nc.sync.dma_start(rstd_d[0:1, :], mu)
            nc.sync.dma_start(rstd_d[1:2, :], msq)
            nc.sync.dma_start(
                cpack[:, 0:2 * NDT].rearrange("p (s tt) -> p s tt", s=2),
                rstd_d[0:2, :].rearrange("s (tt p) -> p s tt", p=128))
            nc.vector.memset(eps_t, LN_EPS)
            nc.vector.scalar_tensor_tensor(var_col, mu_col, -1.0, mu_col,
                                           OP.mult, OP.mult)
            nc.vector.tensor_tensor(var_col, msq_col, var_col, OP.add)
            nc.scalar.activation(sd_col, var_col, AF.Sqrt, bias=eps_t)
            nc.vector.reciprocal(rstd_col[:], sd_col)

            # load the fused output weight (deferred: keeps startup DMA free)
            nc.sync.dma_start(
                wcomb_s[:].rearrange("p (ct c) -> p ct c", c=D),
                wcomb.rearrange("(ct p) c -> p ct c", p=128),
            )

            if has_ln_b:
                # extra term z @ (diag(ln_b) @ wcomb), staged via DRAM while
                # z is still pre-apply (rarely used: ln_b is usually zero)
                y2_d = dram.tile([TC, D], F32, tag="y2d")
                with tc.tile_pool(name="plnw", bufs=4) as lwpool, \
                     tc.tile_pool(name="plnp", bufs=4, space="PSUM") as lppool, \
                     tc.tile_pool(name="plno", bufs=4) as lopool:
                    for tt in range(NDT):
                        for cg in range(2):
                            y2ps = lppool.tile([128, 512], F32, tag="y2ps")
                            for ct in range(NCT):
                                wbt = lwpool.tile([128, 512], BF16, tag="wbt")
                                nc.sync.dma_start(
                                    wbt[:],
                                    wcomb_b[ct * 128:(ct + 1) * 128,
                                            cg * 512:(cg + 1) * 512])
                                nc.tensor.matmul(
                                    y2ps[:],
                                    z_s[:, ct * TC + tt * 128:
                                        ct * TC + (tt + 1) * 128],
                                    wbt[:],
                                    start=(ct == 0), stop=(ct == NCT - 1))
                            y2sb = lopool.tile([128, 512], F32, tag="y2sb")
                            nc.scalar.activation(y2sb[:], y2ps[:], AF.Copy)
                            nc.sync.dma_start(
                                y2_d[tt * 128:(tt + 1) * 128,
                                     cg * 512:(cg + 1) * 512], y2sb[:])

            # ===== phase 4+5 interleaved: LN apply feeding output matmuls =====
            # cg=0 accumulates per-ct as apply results land; cg=1 runs dense.
            p45_bufs = 3 if (has_out_bias or has_ln_b) else 4
            with tc.tile_pool(name="p45f", bufs=p45_bufs) as p4pool, \
                 tc.tile_pool(name="p5ps", bufs=8, space="PSUM") as ypool, \
                 tc.tile_pool(name="p5o", bufs=4) as opool:

                def evict_tt(tt, cg, y_ps):
                    out_sb = opool.tile([128, 512], F32, tag="osb")
                    nc.scalar.activation(out_sb[:], y_ps[:], AF.Copy,
                                         scale=rstd_col[:, tt:tt + 1])
                    if has_ln_b:
                        y2sb = opool.tile([128, 512], F32, tag="y2r")
                        nc.sync.dma_start(
                            y2sb[:], y2_d[tt * 128:(tt + 1) * 128,
                                          cg * 512:(cg + 1) * 512])
                        nc.vector.tensor_tensor(out_sb[:], out_sb[:],
                                                y2sb[:], OP.add)
                    if has_out_bias:
                        nc.vector.tensor_tensor(
                            out_sb[:], out_sb[:],
                            obias_s[:, cg * 512:(cg + 1) * 512], OP.add)
                    nc.sync.dma_start(
                        out[tt * 128:(tt + 1) * 128, cg * 512:(cg + 1) * 512],
                        out_sb[:])

                ytiles = []
                for _tt in range(NDT):
                    ybank = ypool.tile([128, 512], F32, tag="y")
                    ytiles.append(ybank)
                for ct in range(NCT):
                    sl = slice(ct * TC, (ct + 1) * TC)
                    t1 = p4pool.tile([128, TC], F32, tag="t1")
                    eng = nc.gpsimd if ct % 3 == 2 else nc.vector
                    eng.tensor_tensor(t1[:], h_s[:, sl], mu_sb[:], OP.subtract)
                    nc.vector.scalar_tensor_tensor(z_s[:, sl], t1[:],
                                                   lng_s[:, ct:ct + 1], z_s[:, sl],
                                                   OP.mult, OP.mult)
                    for tt in range(NDT):
                        nc.tensor.matmul(
                            ytiles[tt][:],
                            z_s[:, ct * TC + tt * 128: ct * TC + (tt + 1) * 128],
                            wcomb_s[:, ct * D: ct * D + 512],
                            start=(ct == 0), stop=(ct == NCT - 1))
                for tt in range(NDT):
                    evict_tt(tt, 0, ytiles[tt])

                for tt in range(NDT):
                    y_ps = ypool.tile([128, 512], F32, tag="y")
                    for ct in range(NCT):
                        nc.tensor.matmul(
                            y_ps[:],
                            z_s[:, ct * TC + tt * 128: ct * TC + (tt + 1) * 128],
                            wcomb_s[:, ct * D + 512: ct * D + 1024],
                            start=(ct == 0), stop=(ct == NCT - 1))
                    evict_tt(tt, 1, y_ps)

        for _rep in range(repeat):
            emit_body()

    nc.compile()
    return nc


@functools.lru_cache(maxsize=4)
def _program(flags):
    return _build_program(*flags)


def _v2m(v):
    return np.ascontiguousarray(np.asarray(v, np.float32).reshape(NCT, 128).T)


def kernel(x, W_ih_w, W_ih_b, W_gate_w, W_gate_b, W_decay_w, W_decay_b,
           W_z_w, W_z_b, conv_w, conv_b, ln_g, ln_b, W_out_w, W_out_b,
           out_w, out_b):
    f32 = np.float32
    bf = ml_dtypes.bfloat16
    x = np.asarray(x, f32)

    out_bias_eff = (np.asarray(W_out_b, f32) @ np.asarray(out_w, f32)
                    + np.asarray(out_b, f32))
    has_ob = bool(np.any(out_bias_eff != 0.0))
    has_lb = bool(np.any(np.asarray(ln_b) != 0.0))
    nc = _program((has_ob, has_lb))

    w4 = np.concatenate([np.asarray(W_ih_w, f32), np.asarray(W_gate_w, f32),
                         np.asarray(W_decay_w, f32), np.asarray(W_z_w, f32)],
                        axis=1).astype(bf)
    wcomb = (np.asarray(W_out_w, f32) @ np.asarray(out_w, f32)).astype(bf)
    convw_m = np.ascontiguousarray(
        np.asarray(conv_w, f32).reshape(NCT, 128, 4).transpose(1, 0, 2)
        .reshape(128, NCT * 4))
    bias_ih_m = _v2m(W_ih_b)
    shared = dict(
        w4=w4, wcomb=wcomb, bias_ih=bias_ih_m,
        bias_gneg=_v2m(-np.asarray(W_gate_b, f32)),
        bias_dec=_v2m(W_decay_b), bias_z=_v2m(W_z_b), conv_b=_v2m(conv_b),
        ln_g=_v2m(ln_g), convw=convw_m)
    if has_ob:
        shared["out_bias"] = np.ascontiguousarray(
            np.tile(out_bias_eff[None, :], (128, 1)).astype(bf))
    if has_lb:
        shared["wcomb_b"] = np.ascontiguousarray(
            (np.asarray(ln_b, f32)[:, None]
             * (np.asarray(W_out_w, f32) @ np.asarray(out_w, f32))).astype(bf))

    zero_halo = np.zeros((128, NCT), f32)
    in_maps = []
    for c in range(NCORES):
        b, k = divmod(c, KCHUNKS)
        t0 = k * TC
        if k == 0:
            xc = np.vstack([np.zeros((HALO, D), f32), x[b, :TC]])
        else:
            xc = x[b, t0 - HALO: t0 + TC]
        xTc = np.ascontiguousarray(xc.T).astype(bf)
        oh = np.zeros(NCORES, f32)
        if k > 0:
            oh[b * KCHUNKS + k - 1] = 1.0
        in_maps.append({
            **shared,
            "xT": xTc,
            "bias_halo": bias_ih_m if k > 0 else zero_halo,
            "onehot": np.ascontiguousarray(np.tile(oh[None, :], (128, 1))),
        })

    res = run_bass_kernel_spmd(nc, in_maps, core_ids=list(range(NCORES)))

    outf = np.empty((B, T, D), f32)
    for c in range(NCORES):
        b, k = divmod(c, KCHUNKS)
        outf[b, k * TC:(k + 1) * TC, :] = res.results[c]["out"]
    return outf



# revision 88
# speedup vs baseline: 1.3406x; 1.3406x over previous
"""Self-contained Trainium2 Bass kernel for nn_MultiHeadMPSRecurrence.

Reference computation (B=2, T=4096, D_MODEL=1024, D_HIDDEN=2048, K=4):
    ih    = causal_depthwise_conv(x @ W_ih + b, conv_w, conv_b)
    gate  = sigmoid(x @ W_gate + b)
    a     = sigmoid(x @ W_decay + b)
    z     = silu(x @ W_z + b)
    alpha = (1-gate)*a ; beta = gate*ih
    h     = scan(h_t = alpha_t*h_{t-1} + beta_t)
    out   = ((z * layernorm(h)) @ W_out + b) @ out_w + b

Sharding: 8 cores = 2 batches x 4 time-chunks of 1024 steps.  Each core runs
the full pipeline on its chunk; the sequential scan is chunk-linked through
AllGathers of per-chunk (prod(alpha), last-local-state) vectors followed by an
on-device prefix combine; each core then fixes its local scan with
h = h0 + cumprod(alpha) * h_in (no re-scan, no DRAM spill).

Matmul precision: the input projections run on the PE in fp8e4 DoubleRow mode
(256-deep contraction per instruction, double-pumped):
  - decay:  plain fp8 (x_hi @ Wd)                    -- sigmoid saturates, error-free
  - gate:   x_hi @ (Wg_hi + Wg_lo)                   -- weight split recovers precision
  - ih, z:  x_hi@W_hi + x_hi@W_lo + x_lo@W_hi        -- both-operand split, ~bf16 exact
All hi/lo splits share one power-of-2 scale so terms accumulate in one PSUM
group; the 1/8192 descale folds into the eviction activation.  The output
matmul (zn @ (W_out@out_w)) stays bf16, with the fused weight streamed from
DRAM during the output phase.  All phase-1 activations (sigmoid/identity/copy/
square) live in one activation-table set; silu is computed as v*sigmoid(v) to
avoid per-tile table reloads.

On-chip layout: channels on partitions (16 tiles of 128ch), time on free dim.
"""

import functools
from contextlib import ExitStack

import ml_dtypes
import numpy as np

import concourse.tile as tile
from concourse import bacc, mybir
from concourse.bass_utils import run_bass_kernel_spmd

BF16 = mybir.dt.bfloat16
F8 = mybir.dt.float8e4
F32 = mybir.dt.float32
AF = mybir.ActivationFunctionType
OP = mybir.AluOpType
AX = mybir.AxisListType
DR = mybir.MatmulPerfMode.DoubleRow

B, T, D, H = 2, 4096, 1024, 2048
NCORES = 8
KCHUNKS = 4          # time chunks per batch
TC = T // KCHUNKS    # 1024 timesteps per core
HALO = 3             # conv taps reaching back in time (K-1)
TCX = TC + HALO + 1  # 1028: +1 pad column so the DoubleRow moving pair
                     # stride (TCX bytes, fp8) stays even -- odd strides
                     # crash the double-pumped moving fetch on hardware
NCT = H // 128       # 16 channel tiles
NDT = D // 128       # 8 d_model tiles
NDP = NDT // 2       # 4 contraction pair-tiles (DoubleRow)
LN_EPS = 1e-5
# channel-tile groups per boundary exchange: earlier groups' post-exchange
# work overlaps the remaining phase-1 iterations (~1 fixup+stats per
# iteration keeps DVE/Act under the PE pace)
GRPS = ((0, 5), (5, 9), (9, 14), (14, 16))

SX = 16.0            # fp8 scale on x
SW = 512.0           # fp8 scale on projection weights
PSCALE = 1.0 / (SX * SW)

# weight tile section ids (per 128-channel tile): 7 fp8 sections of NDT*128
S_IHH, S_IHL, S_GH, S_GL, S_DEC, S_ZH, S_ZL = range(7)
NSEC = 7


def _build_program(has_out_bias: bool, has_ln_b: bool, sim_no_cc: bool = False,
                   repeat: int = 1, _stage: str = "full"):
    nc = bacc.Bacc("TRN2", target_bir_lowering=False, debug=False,
                   enable_asserts=True, num_devices=NCORES)

    # ---------------- I/O ----------------
    xh_d = nc.dram_tensor("xh", [D, TCX], F8, kind="ExternalInput").ap()
    xl_d = nc.dram_tensor("xl", [D, TCX], F8, kind="ExternalInput").ap()
    w7 = nc.dram_tensor("w7", [D, NCT * NSEC * 128], F8,
                        kind="ExternalInput").ap()
    wcomb = nc.dram_tensor("wcomb", [H, D], BF16, kind="ExternalInput").ap()
    # per-channel vectors in [partition, ch_tile] layout
    def cvec(name, n=NCT):
        return nc.dram_tensor(name, [128, n], F32, kind="ExternalInput").ap()
    bias_ih = cvec("bias_ih")
    bias_halo = cvec("bias_halo")
    bias_gneg = cvec("bias_gneg")
    bias_dec = cvec("bias_dec")
    bias_z = cvec("bias_z")
    conv_b_d = cvec("conv_b")
    ln_g_d = cvec("ln_g")
    convw_d = cvec("convw", NCT * 4)
    onehot_d = cvec("onehot", NCORES)
    if has_out_bias:
        out_bias_d = nc.dram_tensor("out_bias", [128, D], BF16,
                                    kind="ExternalInput").ap()
    if has_ln_b:
        wcomb_b = nc.dram_tensor("wcomb_b", [H, D], BF16,
                                 kind="ExternalInput").ap()
    out = nc.dram_tensor("out", [TC, D], F32, kind="ExternalOutput").ap()

    with tile.TileContext(nc) as tc, ExitStack() as ctx:
        dram = ctx.enter_context(tc.tile_pool(name="dram", bufs=1, space="DRAM"))
        rstd_d = dram.tile([2, TC], F32)

        # ---------------- constants / small resident ----------------
        consts = ctx.enter_context(tc.tile_pool(name="consts", bufs=1))

        def load_const(name, ap_in, n):
            # Act-queue DMAs: keep the SP queue free for the startup x/w loads
            t = consts.tile([128, n], F32, tag=name)
            nc.scalar.dma_start(t[:], ap_in[:, :])
            return t
        bih_s = load_const("bih", bias_ih, NCT)
        bhalo_s = load_const("bhalo", bias_halo, NCT)
        bgn_s = load_const("bgn", bias_gneg, NCT)
        bdec_s = load_const("bdec", bias_dec, NCT)
        bz_s = load_const("bz", bias_z, NCT)
        cb_s = load_const("cb", conv_b_d, NCT)
        lng_s = load_const("lng", ln_g_d, NCT)
        cw_s = load_const("cw", convw_d, NCT * 4)
        oh_s = load_const("oh", onehot_d, NCORES)
        if has_out_bias:
            obias_s = consts.tile([128, D], BF16, tag="obias")
            nc.sync.dma_start(obias_s[:], out_bias_d[:, :])
        ones_stat = consts.tile([128, 1], BF16)
        nc.vector.memset(ones_stat[:], 1.0)
        ones_bc = consts.tile([1, 128], BF16)
        nc.vector.memset(ones_bc[:], 1.0)
        one_f32 = consts.tile([1, 1], F32)
        nc.vector.memset(one_f32[:], 1.0)

        # ---------------- big resident tiles ----------------
        res = ctx.enter_context(tc.tile_pool(name="res", bufs=1))
        z_s = res.tile([128, NCT * TC], BF16)
        h_s = res.tile([128, NCT * TC], BF16)
        cumA_s = res.tile([128, NCT * TC], BF16)
        rows = ctx.enter_context(tc.tile_pool(name="rows", bufs=1))
        Pbuf = res.tile([128, NCT], F32)
        Lbuf = res.tile([128, NCT], F32)
        h_in = res.tile([128, NCT], F32)
        # gathered (P,L) pairs, layout [p, ct, core j, {P,L}]
        PL = res.tile([128, NCT * NCORES * 2], F32)
        Hp = res.tile([128, NCT * NCORES], F32)
        rstd_col = res.tile([128, NDT], F32)
        # second half of the fused output weight, resident for the tt-outer
        # cg=1 pass (loaded during the exchange tail)
        wc1_s = res.tile([128, NCT * 512], BF16)
        mu_sb = rows.tile([128, TC], BF16)
        mu_row = rows.tile([1, TC], F32)
        msq_row = rows.tile([1, TC], F32)
        mu_bf = rows.tile([1, TC], BF16)
        wc0head = rows.tile([128, 2 * 512], BF16)
        mu = mu_row[:, :]
        msq = msq_row[:, :]
        cpack = rows.tile([128, 4 * NDT + 1], F32)
        mu_col = cpack[:, 0:NDT]
        msq_col = cpack[:, NDT:2 * NDT]
        var_col = cpack[:, 2 * NDT:3 * NDT]
        sd_col = cpack[:, 3 * NDT:4 * NDT]
        eps_t = cpack[:, 4 * NDT:4 * NDT + 1]

        def emit_body():
            cc_bufs = []
            for gi, (lo, hi) in enumerate(GRPS):
                n = hi - lo
                cci = dram.tile([1, 2 * n * 128], F32, tag=f"cc{gi}i",
                                name=f"cc{gi}i")
                cco = dram.tile([NCORES, 2 * n * 128], F32,
                                addr_space="Shared", tag=f"cc{gi}o",
                                name=f"cc{gi}o")
                cc_bufs.append((cci, cco))

            def comm_grp(gi):
                """Per-batch AllGather of (P,L) for channel tiles [lo,hi);
                prefix-combine (one masked scan over the (ct, core) sequence)
                and select this core's incoming state into h_in[:, lo:hi]."""
                lo, hi = GRPS[gi]
                cci, cco = cc_bufs[gi]
                n = hi - lo
                # payload layout (ct, p, {P,L}) so each per-core readback is
                # one 3-dim DMA with contiguous innermost pairs
                # the final exchange is latency-critical and runs when the Act
                # queue is idle; earlier exchanges keep Act free for evictions
                last = (gi == len(GRPS) - 1)
                deng = nc.scalar if last else nc.sync
                cciV = cci[0:1, :].rearrange("o (ct p pl) -> p (o ct) pl",
                                             p=128, pl=2)
                deng.dma_start(cciV[:, :, 0], Pbuf[:, lo:hi])
                deng.dma_start(cciV[:, :, 1], Lbuf[:, lo:hi])
                if sim_no_cc:
                    # TimelineSim can't model collectives; local DMA stand-in.
                    deng.dma_start(cco[0:1, :], cci[:, :])
                else:
                    nc.gpsimd.collective_compute(
                        "AllGather", OP.bypass,
                        replica_groups=[list(range(NCORES))],
                        ins=[cci.opt()], outs=[cco.opt()])
                PLV = PL[:].rearrange("p (ct j pl) -> p ct j pl",
                                      j=NCORES, pl=2)
                for j in range(NCORES):
                    eng = deng
                    eng.dma_start(
                        PLV[:, lo:hi, j, :],
                        cco[j:j + 1, :].rearrange("o (ct p pl) -> p (o ct) pl",
                                                  p=128, pl=2))
                # zero the P entries at each batch-start core: the combined
                # scan then resets there, so one linear scan over the whole
                # (ct, core) sequence computes every prefix state
                for j in range(0, NCORES, KCHUNKS):
                    nc.vector.memset(PLV[:, lo:hi, j, 0], 0.0)
                HpV = Hp[:].rearrange("p (ct j) -> p ct j", j=NCORES)
                PLf = (PL[:, lo * 2 * NCORES:hi * 2 * NCORES]
                       .rearrange("p (k pl) -> p k pl", pl=2))
                nc.vector.tensor_tensor_scan(
                    Hp[:, lo * NCORES:hi * NCORES], PLf[:, :, 0],
                    PLf[:, :, 1], 0.0, OP.mult, OP.add)
                nc.vector.memset(h_in[:, lo:hi], 0.0)
                for j in range(NCORES):
                    nc.vector.scalar_tensor_tensor(
                        h_in[:, lo:hi], HpV[:, lo:hi, j], oh_s[:, j:j + 1],
                        h_in[:, lo:hi], OP.mult, OP.add)

            # schedule each group's fixup+stats into the iterations between
            # its exchange and the next one (last group lands in the tail);
            # keep the final iteration free so its own chain finishes early
            p3_sched = {}
            for gi, (lo, hi) in enumerate(GRPS[:-1]):
                slot_lo, slot_hi = GRPS[gi][1], GRPS[gi + 1][1]
                nslots = slot_hi - slot_lo
                for i, ct in enumerate(range(lo, hi)):
                    slot = slot_lo + min(i * nslots // (hi - lo), nslots - 1)
                    p3_sched.setdefault(slot, []).append(ct)

            with tc.tile_pool(name="p3ps", bufs=1, space="PSUM") as spool, \
                 tc.tile_pool(name="p3b", bufs=2) as p3pool:
                # one psum bank holds all four stat rows: sum(h) halves on
                # partitions 0/64, sum(h^2) halves on partitions 32/96
                psum_st = spool.tile([128, 512], F32)

                def fixup_ct(ct):
                    """h = h0 + cumA * h_in for one channel tile."""
                    sl = slice(ct * TC, (ct + 1) * TC)
                    nc.vector.scalar_tensor_tensor(
                        h_s[:, sl], cumA_s[:, sl], h_in[:, ct:ct + 1],
                        h_s[:, sl], OP.mult, OP.add)

                def stats_ct(ct):
                    """accumulate sum(h), sum(h^2) over channels via PE."""
                    sl = slice(ct * TC, (ct + 1) * TC)
                    h2 = p3pool.tile([128, TC], BF16, tag="h2")
                    if ct % 2 == 0:
                        nc.vector.tensor_tensor(h2[:], h_s[:, sl], h_s[:, sl],
                                                OP.mult)
                    else:
                        nc.scalar.activation(h2[:], h_s[:, sl], AF.Square)
                    for half in range(2):
                        nc.tensor.matmul(
                            psum_st[64 * half:64 * half + 1, 0:512],
                            ones_stat[:, 0:1],
                            h_s[:, ct * TC + half * 512:
                                ct * TC + (half + 1) * 512],
                            start=(ct == 0), stop=(ct == NCT - 1),
                            tile_position=(0, 64 * half))
                    for half in range(2):
                        nc.tensor.matmul(
                            psum_st[64 * half + 32:64 * half + 33, 0:512],
                            ones_stat[:, 0:1],
                            h2[:, half * 512:(half + 1) * 512],
                            start=(ct == 0), stop=(ct == NCT - 1),
                            tile_position=(0, 64 * half + 32))

                # ============ phase 1: matmuls + gates + local scans ==========
                with tc.tile_pool(name="p1xt", bufs=1) as xtpool, \
                     tc.tile_pool(name="p1w", bufs=2) as wpool, \
                     tc.tile_pool(name="p1ps", bufs=3, space="PSUM") as ppool, \
                     tc.tile_pool(name="p1hs", bufs=1, space="PSUM") as hpool, \
                     tc.tile_pool(name="p1f", bufs=2) as fpool, \
                     tc.tile_pool(name="p1b", bufs=2) as bpool:
                    xh_s = xtpool.tile([128, NDT * TCX], F8)
                    xl_s = xtpool.tile([128, NDT * TCX], F8)

                    def load_w(ct, split=False):
                        # SBUF layout [p, dt, s, c]: per-dt blocks of 7
                        # contiguous 128-col sections (3-dim DMA both sides)
                        w_t = wpool.tile([128, NSEC * NDT * 128], F8, tag="w_t")
                        dst = w_t[:].rearrange("p (dt sc) -> p dt sc",
                                               sc=NSEC * 128)
                        src = (w7[:, ct * NSEC * 128:(ct + 1) * NSEC * 128]
                               .rearrange("(dt p) sc -> p dt sc", p=128))
                        if split:
                            # ih sections first so m=0 can start sooner
                            nc.sync.dma_start(dst[:, :, 0:256], src[:, :, 0:256])
                            nc.sync.dma_start(dst[:, :, 256:NSEC * 128],
                                              src[:, :, 256:NSEC * 128])
                        else:
                            nc.sync.dma_start(dst, src)
                        return w_t

                    # startup order (DMA transfers serialize): ih weight
                    # sections, then x hi/lo in one transfer each, then the
                    # rest of the ct=0 weights
                    w_next = wpool.tile([128, NSEC * NDT * 128], F8, tag="w_t",
                                        name="w_t0")
                    dst0 = w_next[:].rearrange("p (dt sc) -> p dt sc",
                                               sc=NSEC * 128)
                    src0 = w7[:, 0:NSEC * 128].rearrange(
                        "(dt p) sc -> p dt sc", p=128)
                    nc.sync.dma_start(dst0[:, :, 0:256], src0[:, :, 0:256])
                    nc.sync.dma_start(
                        xh_s[:].rearrange("p (dt t) -> p dt t", t=TCX),
                        xh_d[:, :].rearrange("(dt p) t -> p dt t", p=128))
                    nc.sync.dma_start(
                        xl_s[:].rearrange("p (dt t) -> p dt t", t=TCX),
                        xl_d[:, :].rearrange("(dt p) t -> p dt t", p=128))
                    nc.sync.dma_start(dst0[:, :, 256:NSEC * 128],
                                      src0[:, :, 256:NSEC * 128])
                    xhv = xh_s[:].rearrange("p (dt t) -> p dt t", t=TCX)
                    xlv = xl_s[:].rearrange("p (dt t) -> p dt t", t=TCX)

                    for ct in range(NCT):
                        w_t = w_next
                        if ct + 1 < NCT:
                            w_next = load_w(ct + 1)
                        wv = w_t[:].rearrange("p (dt s c) -> p s dt c",
                                              s=NSEC, c=128)

                        def mmgroup(terms, lo, n, ps_ap):
                            """terms: list of (section, x-view); all accumulate
                            into ps_ap as one PSUM group (shared scale)."""
                            ni = len(terms) * NDP
                            i = 0
                            for (s, xv) in terms:
                                for k in range(NDP):
                                    nc.tensor.matmul(
                                        ps_ap,
                                        wv[:, s, 2 * k:2 * k + 2, :],
                                        xv[:, 2 * k:2 * k + 2, lo:lo + n],
                                        start=(i == 0), stop=(i == ni - 1),
                                        perf_mode=DR)
                                    i += 1

                        T_IH = [(S_IHH, xhv), (S_IHL, xhv), (S_IHH, xlv)]
                        T_G = [(S_GH, xhv), (S_GL, xhv)]
                        T_D = [(S_DEC, xhv)]
                        T_Z = [(S_ZH, xhv), (S_ZL, xhv), (S_ZH, xlv)]

                        # m=0: ih over 1027 halo columns -> bf16 ih_pre
                        ih_pre = fpool.tile([128, TCX], BF16, tag="ihpre")
                        ps_halo = hpool.tile([128, HALO], F32, tag="pshalo")
                        mmgroup(T_IH, 0, HALO, ps_halo[:, :])
                        nc.scalar.activation(ih_pre[:, 0:HALO], ps_halo[:, :],
                                             AF.Identity,
                                             bias=bhalo_s[:, ct:ct + 1],
                                             scale=PSCALE)
                        ps = ppool.tile([128, TC], F32, tag="ps")
                        for half in range(2):
                            mmgroup(T_IH, HALO + half * 512, 512,
                                    ps[:, half * 512:(half + 1) * 512])
                        nc.scalar.activation(ih_pre[:, HALO:HALO + TC], ps[:],
                                             AF.Identity,
                                             bias=bih_s[:, ct:ct + 1],
                                             scale=PSCALE)
                        # m=1: gp = sigmoid(-(x@Wg + bg)) = 1-gate
                        gp = fpool.tile([128, TC], BF16, tag="gp")
                        ps = ppool.tile([128, TC], F32, tag="ps")
                        for half in range(2):
                            mmgroup(T_G, HALO + half * 512, 512,
                                    ps[:, half * 512:(half + 1) * 512])
                        nc.scalar.activation(gp[:], ps[:], AF.Sigmoid,
                                             bias=bgn_s[:, ct:ct + 1],
                                             scale=-PSCALE)
                        # m=2: a = sigmoid(x@Wd + bd)
                        a_t = fpool.tile([128, TC], BF16, tag="a")
                        ps = ppool.tile([128, TC], F32, tag="ps")
                        for half in range(2):
                            mmgroup(T_D, HALO + half * 512, 512,
                                    ps[:, half * 512:(half + 1) * 512])
                        nc.scalar.activation(a_t[:], ps[:], AF.Sigmoid,
                                             bias=bdec_s[:, ct:ct + 1],
                                             scale=PSCALE)
                        # m=3: z = silu(v) = v * sigmoid(v), avoiding the silu
                        # activation table (stays in the sigmoid table set)
                        ps = ppool.tile([128, TC], F32, tag="ps")
                        for half in range(2):
                            mmgroup(T_Z, HALO + half * 512, 512,
                                    ps[:, half * 512:(half + 1) * 512])
                        zv = fpool.tile([128, TC], BF16, tag="zv")
                        nc.scalar.activation(zv[:], ps[:], AF.Identity,
                                             bias=bz_s[:, ct:ct + 1],
                                             scale=PSCALE)
                        zsg = fpool.tile([128, TC], BF16, tag="zsg")
                        nc.scalar.activation(zsg[:], ps[:], AF.Sigmoid,
                                             bias=bz_s[:, ct:ct + 1],
                                             scale=PSCALE)
                        nc.gpsimd.tensor_tensor(z_s[:, ct * TC:(ct + 1) * TC],
                                                zv[:], zsg[:], OP.mult)

                        # causal depthwise conv (4 taps, halo in ih_pre)
                        ihc = fpool.tile([128, TC], BF16, tag="ihc")
                        nc.scalar.activation(ihc[:], ih_pre[:, 3:3 + TC],
                                             AF.Identity,
                                             bias=cb_s[:, ct:ct + 1],
                                             scale=cw_s[:, ct * 4 + 3: ct * 4 + 4])
                        for j in (2, 1, 0):
                            nc.vector.scalar_tensor_tensor(
                                ihc[:], ih_pre[:, j:j + TC],
                                cw_s[:, ct * 4 + j: ct * 4 + j + 1],
                                ihc[:], OP.mult, OP.add)

                        # alpha = gp*a ; gate = 1-gp ; beta = gate*ihc
                        alpha_t = bpool.tile([128, TC], BF16, tag="alpha")
                        nc.gpsimd.tensor_tensor(alpha_t[:], gp[:], a_t[:],
                                                OP.mult)
                        gate_t = bpool.tile([128, TC], BF16, tag="gate")
                        nc.scalar.activation(gate_t[:], gp[:], AF.Copy,
                                             bias=1.0, scale=-1.0)
                        beta_t = bpool.tile([128, TC], BF16, tag="beta")
                        nc.gpsimd.tensor_tensor(beta_t[:], gate_t[:], ihc[:],
                                                OP.mult)

                        # local scan (init 0) -> h_s ; cumprod(alpha) -> cumA_s
                        hsl = h_s[:, ct * TC:(ct + 1) * TC]
                        nc.vector.tensor_tensor_scan(hsl, alpha_t[:],
                                                     beta_t[:], 0.0,
                                                     OP.mult, OP.add)
                        casl = cumA_s[:, ct * TC:(ct + 1) * TC]
                        nc.vector.tensor_tensor_scan(casl, alpha_t[:],
                                                     alpha_t[:], 1.0,
                                                     OP.mult, OP.bypass)
                        # last-column extracts ride the Pool queue (it has
                        # slack) so the exchange writes never wait behind the
                        # DVE backlog
                        nc.gpsimd.tensor_copy(Lbuf[:, ct:ct + 1],
                                              hsl[:, TC - 1:TC])
                        nc.gpsimd.tensor_copy(Pbuf[:, ct:ct + 1],
                                              casl[:, TC - 1:TC])

                        if _stage != "p1":
                            for gi, (glo, ghi) in enumerate(GRPS[:-1]):
                                if ct == ghi - 1:
                                    comm_grp(gi)
                            for c3 in p3_sched.get(ct, []):
                                fixup_ct(c3)
                                stats_ct(c3)

                if _stage == "p1":
                    with tc.tile_pool(name="dbg", bufs=1) as dbgp:
                        dbg = dbgp.tile([128, TC], F32)
                        nc.scalar.activation(dbg[:], h_s[:, 0:TC], AF.Copy)
                        nc.sync.dma_start(out[0:128, 0:TC], dbg[:])
                    return

                # ============ last boundary exchange + fixups =================
                comm_grp(len(GRPS) - 1)
                # load the cg=1 output-weight half on-chip during the tail so
                # the final output pass never touches DRAM (after the exchange
                # DMAs -- transfers serialize on the DMA engines)
                wc1v = wc1_s[:].rearrange("p (ct c) -> p ct c", c=512)
                nc.sync.dma_start(
                    wc1v[:, :, :],
                    wcomb[:, 512:1024].rearrange("(ct p) c -> p ct c", p=128))
                nc.sync.dma_start(wc0head[:].rearrange("p (ct c) -> p ct c",
                                                       c=512),
                                  wcomb[0:256, 0:512].rearrange(
                                      "(ct p) c -> p ct c", p=128))
                for ct in range(GRPS[-1][0], NCT):
                    fixup_ct(ct)
                    stats_ct(ct)

                # ============ LN parameters ===================================
                # bf16 mu first: it alone gates the apply phase (broadcast);
                # the f32 rows only feed the rstd chain for the evictions
                for half in range(2):
                    hs = slice(half * 512, (half + 1) * 512)
                    nc.scalar.activation(mu_bf[0:1, hs],
                                         psum_st[64 * half:64 * half + 1, :],
                                         AF.Copy, scale=1.0 / H)
                for half in range(2):
                    hs = slice(half * 512, (half + 1) * 512)
                    nc.scalar.activation(mu[:, hs],
                                         psum_st[64 * half:64 * half + 1, :],
                                         AF.Copy, scale=1.0 / H)
                    nc.vector.tensor_scalar(
                        msq[:, hs],
                        psum_st[64 * half + 32:64 * half + 33, :],
                        1.0 / H, None, OP.mult)

            if _stage == "p3":
                with tc.tile_pool(name="dbg", bufs=1) as dbgp:
                    dbg = dbgp.tile([128, TC], F32)
                    nc.scalar.activation(dbg[:], h_s[:, 0:TC], AF.Copy)
                    nc.sync.dma_start(out[0:128, 0:TC], dbg[:])
                    nc.sync.dma_start(out[128:129, 0:TC], mu)
                return

            # broadcast mu first: it gates the apply phase; the rstd
            # column chain only gates the final evictions
            with tc.tile_pool(name="p4ps", bufs=1, space="PSUM") as bpool4:
                mu_b = bpool4.tile([128, TC], F32)
                for half in range(2):
                    nc.tensor.matmul(mu_b[:, half * 512:(half + 1) * 512],
                                     ones_bc[0:1, :],
                                     mu_bf[0:1, half * 512:(half + 1) * 512])
                nc.scalar.activation(mu_sb[:], mu_b[:], AF.Copy)
                # transpose the f32 stat rows into [tt-column] layout on the
                # PE (replaces a DRAM round-trip)
                tps = bpool4.tile([128, 2 * NDT], F32)
                for s, row in ((0, mu), (1, msq)):
                    for tt in range(NDT):
                        nc.tensor.matmul(
                            tps[:, s * NDT + tt:s * NDT + tt + 1],
                            row[0:1, tt * 128:(tt + 1) * 128],
                            one_f32[0:1, 0:1], is_transpose=True)
                nc.scalar.activation(cpack[:, 0:2 * NDT], tps[:], AF.Copy)
            nc.vector.memset(eps_t, LN_EPS)
            nc.vector.scalar_tensor_tensor(var_col, mu_col, -1.0, mu_col,
                                           OP.mult, OP.mult)
            nc.vector.tensor_tensor(var_col, msq_col, var_col, OP.add)
            nc.scalar.activation(sd_col, var_col, AF.Sqrt, bias=eps_t)
            nc.vector.reciprocal(rstd_col[:], sd_col)

            if has_ln_b:
                # extra term z @ (diag(ln_b) @ wcomb), staged via DRAM while
                # z is still pre-apply (rarely used: ln_b is usually zero)
                y2_d = dram.tile([TC, D], F32, tag="y2d")
                with tc.tile_pool(name="plnw", bufs=4) as lwpool, \
                     tc.tile_pool(name="plnp", bufs=4, space="PSUM") as lppool, \
                     tc.tile_pool(name="plno", bufs=4) as lopool:
                    for tt in range(NDT):
                        for cg in range(2):
                            y2ps = lppool.tile([128, 512], F32, tag="y2ps")
                            for ct in range(NCT):
                                wbt = lwpool.tile([128, 512], BF16, tag="wbt")
                                nc.sync.dma_start(
                                    wbt[:],
                                    wcomb_b[ct * 128:(ct + 1) * 128,
                                            cg * 512:(cg + 1) * 512])
                                nc.tensor.matmul(
                                    y2ps[:],
                                    z_s[:, ct * TC + tt * 128:
                                        ct * TC + (tt + 1) * 128],
                                    wbt[:],
                                    start=(ct == 0), stop=(ct == NCT - 1))
                            y2sb = lopool.tile([128, 512], F32, tag="y2sb")
                            nc.scalar.activation(y2sb[:], y2ps[:], AF.Copy)
                            nc.sync.dma_start(
                                y2_d[tt * 128:(tt + 1) * 128,
                                     cg * 512:(cg + 1) * 512], y2sb[:])

            # ===== phase 4+5 interleaved: LN apply feeding output matmuls =====
            # cg=0 accumulates per-ct as apply results land; cg=1 runs
            # tt-outer so each output block evicts as soon as it completes.
            with tc.tile_pool(name="p45f", bufs=3) as p4pool, \
                 tc.tile_pool(name="p45w", bufs=3) as wspool, \
                 tc.tile_pool(name="p5ps", bufs=8, space="PSUM") as ypool, \
                 tc.tile_pool(name="p5o", bufs=4) as opool:

                def wslice(ct):
                    w_sl = wspool.tile([128, 512], BF16, tag="wsl")
                    nc.sync.dma_start(
                        w_sl[:], wcomb[ct * 128:(ct + 1) * 128, 0:512])
                    return w_sl

                def evict_tt(tt, cg, y_ps):
                    out_sb = opool.tile([128, 512], F32, tag="osb")
                    if tt % 2 == 0:
                        nc.scalar.activation(out_sb[:], y_ps[:], AF.Copy,
                                             scale=rstd_col[:, tt:tt + 1])
                    else:
                        nc.vector.tensor_scalar(out_sb[:], y_ps[:],
                                                rstd_col[:, tt:tt + 1], None,
                                                OP.mult)
                    if has_ln_b:
                        y2sb = opool.tile([128, 512], F32, tag="y2r")
                        nc.sync.dma_start(
                            y2sb[:], y2_d[tt * 128:(tt + 1) * 128,
                                          cg * 512:(cg + 1) * 512])
                        nc.vector.tensor_tensor(out_sb[:], out_sb[:],
                                                y2sb[:], OP.add)
                    if has_out_bias:
                        nc.vector.tensor_tensor(
                            out_sb[:], out_sb[:],
                            obias_s[:, cg * 512:(cg + 1) * 512], OP.add)
                    nc.sync.dma_start(
                        out[tt * 128:(tt + 1) * 128, cg * 512:(cg + 1) * 512],
                        out_sb[:])

                ytiles = []
                for _tt in range(NDT):
                    ybank = ypool.tile([128, 512], F32, tag="y")
                    ytiles.append(ybank)
                for ct in range(NCT):
                    sl = slice(ct * TC, (ct + 1) * TC)
                    w_sl = wc0head[:, ct * 512:(ct + 1) * 512] if ct < 2 \
                        else wslice(ct)[:]
                    t1 = p4pool.tile([128, TC], BF16, tag="t1")
                    nc.vector.tensor_tensor(t1[:], h_s[:, sl], mu_sb[:],
                                            OP.subtract)
                    nc.vector.scalar_tensor_tensor(z_s[:, sl], t1[:],
                                                   lng_s[:, ct:ct + 1],
                                                   z_s[:, sl],
                                                   OP.mult, OP.mult)
                    for tt in range(NDT):
                        nc.tensor.matmul(
                            ytiles[tt][:],
                            z_s[:, ct * TC + tt * 128: ct * TC + (tt + 1) * 128],
                            w_sl,
                            start=(ct == 0), stop=(ct == NCT - 1))
                for tt in range(NDT):
                    evict_tt(tt, 0, ytiles[tt])

                # cg=1: tt-outer; each block evicts right after its last ct
                for tt in range(NDT):
                    y_ps = ypool.tile([128, 512], F32, tag="y")
                    for ct in range(NCT):
                        nc.tensor.matmul(
                            y_ps[:],
                            z_s[:, ct * TC + tt * 128: ct * TC + (tt + 1) * 128],
                            wc1v[:, ct, :],
                            start=(ct == 0), stop=(ct == NCT - 1))
                    evict_tt(tt, 1, y_ps)

        for _rep in range(repeat):
            emit_body()

    nc.compile()
    return nc


@functools.lru_cache(maxsize=4)
def _program(flags):
    return _build_program(*flags)


def _v2m(v):
    return np.ascontiguousarray(np.asarray(v, np.float32).reshape(NCT, 128).T)


F8NP = ml_dtypes.float8_e4m3


def _split8(w, s):
    """split fp32 array into (hi, lo) fp8 parts sharing scale s."""
    hi = (w * s).astype(F8NP)
    lo = ((w - hi.astype(np.float32) / s) * s).astype(F8NP)
    return hi, lo


def kernel(x, W_ih_w, W_ih_b, W_gate_w, W_gate_b, W_decay_w, W_decay_b,
           W_z_w, W_z_b, conv_w, conv_b, ln_g, ln_b, W_out_w, W_out_b,
           out_w, out_b):
    f32 = np.float32
    bf = ml_dtypes.bfloat16
    x = np.asarray(x, f32)

    out_bias_eff = (np.asarray(W_out_b, f32) @ np.asarray(out_w, f32)
                    + np.asarray(out_b, f32))
    has_ob = bool(np.any(out_bias_eff != 0.0))
    has_lb = bool(np.any(np.asarray(ln_b) != 0.0))
    nc = _program((has_ob, has_lb))

    # fp8 weight sections, packed per channel-tile: [D, NCT*7*128]
    ih_h, ih_l = _split8(np.asarray(W_ih_w, f32), SW)
    g_h, g_l = _split8(np.asarray(W_gate_w, f32), SW)
    d_q = (np.asarray(W_decay_w, f32) * SW).astype(F8NP)
    z_h, z_l = _split8(np.asarray(W_z_w, f32), SW)
    sects = np.stack([ih_h, ih_l, g_h, g_l, d_q, z_h, z_l], axis=1)  # [D,7,H]
    w7 = np.ascontiguousarray(
        sects.reshape(D, NSEC, NCT, 128).transpose(0, 2, 1, 3)
        .reshape(D, NCT * NSEC * 128))

    wcomb = (np.asarray(W_out_w, f32) @ np.asarray(out_w, f32)).astype(bf)
    convw_m = np.ascontiguousarray(
        np.asarray(conv_w, f32).reshape(NCT, 128, 4).transpose(1, 0, 2)
        .reshape(128, NCT * 4))
    bias_ih_m = _v2m(W_ih_b)
    shared = dict(
        w7=w7, wcomb=wcomb, bias_ih=bias_ih_m,
        bias_gneg=_v2m(-np.asarray(W_gate_b, f32)),
        bias_dec=_v2m(W_decay_b), bias_z=_v2m(W_z_b), conv_b=_v2m(conv_b),
        ln_g=_v2m(ln_g), convw=convw_m)
    if has_ob:
        shared["out_bias"] = np.ascontiguousarray(
            np.tile(out_bias_eff[None, :], (128, 1)).astype(bf))
    if has_lb:
        shared["wcomb_b"] = np.ascontiguousarray(
            (np.asarray(ln_b, f32)[:, None]
             * (np.asarray(W_out_w, f32) @ np.asarray(out_w, f32))).astype(bf))

    zero_halo = np.zeros((128, NCT), f32)
    in_maps = []
    for c in range(NCORES):
        b, k = divmod(c, KCHUNKS)
        t0 = k * TC
        pad = np.zeros((1, D), f32)
        if k == 0:
            xc = np.vstack([np.zeros((HALO, D), f32), x[b, :TC], pad])
        else:
            xc = np.vstack([x[b, t0 - HALO: t0 + TC], pad])
        xb = np.ascontiguousarray(xc.T).astype(bf).astype(f32)
        xh8, xl8 = _split8(xb, SX)
        oh = np.zeros(NCORES, f32)
        if k > 0:
            oh[b * KCHUNKS + k - 1] = 1.0
        in_maps.append({
            **shared,
            "xh": np.ascontiguousarray(xh8),
            "xl": np.ascontiguousarray(xl8),
            "bias_halo": bias_ih_m if k > 0 else zero_halo,
            "onehot": np.ascontiguousarray(np.tile(oh[None, :], (128, 1))),
        })

    res = run_bass_kernel_spmd(nc, in_maps, core_ids=list(range(NCORES)))

    outf = np.empty((B, T, D), f32)
    for c in range(NCORES):
        b, k = divmod(c, KCHUNKS)
        outf[b, k * TC:(k + 1) * TC, :] = res.results[c]["out"]
    return outf


# revision 91
# speedup vs baseline: 1.3834x; 1.0319x over previous
"""Self-contained Trainium2 Bass kernel for nn_MultiHeadMPSRecurrence.

Reference computation (B=2, T=4096, D_MODEL=1024, D_HIDDEN=2048, K=4):
    ih    = causal_depthwise_conv(x @ W_ih + b, conv_w, conv_b)
    gate  = sigmoid(x @ W_gate + b)
    a     = sigmoid(x @ W_decay + b)
    z     = silu(x @ W_z + b)
    alpha = (1-gate)*a ; beta = gate*ih
    h     = scan(h_t = alpha_t*h_{t-1} + beta_t)
    out   = ((z * layernorm(h)) @ W_out + b) @ out_w + b

Sharding: 8 cores = 2 batches x 4 time-chunks of 1024 steps.  Each core runs
the full pipeline on its chunk; the sequential scan is chunk-linked through
AllGathers of per-chunk (prod(alpha), last-local-state) vectors followed by an
on-device prefix combine; each core then fixes its local scan with
h = h0 + cumprod(alpha) * h_in (no re-scan, no DRAM spill).

Matmul precision: the input projections run on the PE in fp8e4 DoubleRow mode
(256-deep contraction per instruction, double-pumped):
  - decay:  plain fp8 (x_hi @ Wd)                    -- sigmoid saturates, error-free
  - gate:   x_hi @ (Wg_hi + Wg_lo)                   -- weight split recovers precision
  - ih, z:  x_hi@W_hi + x_hi@W_lo + x_lo@W_hi        -- both-operand split, ~bf16 exact
All hi/lo splits share one power-of-2 scale so terms accumulate in one PSUM
group; the 1/8192 descale folds into the eviction activation.  The output
matmul (zn @ (W_out@out_w)) stays bf16, with the fused weight streamed from
DRAM during the output phase.  All phase-1 activations (sigmoid/identity/copy/
square) live in one activation-table set; silu is computed as v*sigmoid(v) to
avoid per-tile table reloads.

On-chip layout: channels on partitions (16 tiles of 128ch), time on free dim.
"""

import functools
from contextlib import ExitStack

import ml_dtypes
import numpy as np

import concourse.tile as tile
from concourse import bacc, mybir
from concourse.bass_utils import run_bass_kernel_spmd

BF16 = mybir.dt.bfloat16
F8 = mybir.dt.float8e4
F32 = mybir.dt.float32
AF = mybir.ActivationFunctionType
OP = mybir.AluOpType
AX = mybir.AxisListType
DR = mybir.MatmulPerfMode.DoubleRow

B, T, D, H = 2, 4096, 1024, 2048
NCORES = 8
KCHUNKS = 4          # time chunks per batch
TC = T // KCHUNKS    # 1024 timesteps per core
HALO = 3             # conv taps reaching back in time (K-1)
TCX = TC + HALO + 1  # 1028: +1 pad column so the DoubleRow moving pair
                     # stride (TCX bytes, fp8) stays even -- odd strides
                     # crash the double-pumped moving fetch on hardware
NCT = H // 128       # 16 channel tiles
NDT = D // 128       # 8 d_model tiles
NDP = NDT // 2       # 4 contraction pair-tiles (DoubleRow)
LN_EPS = 1e-5
# channel-tile groups per boundary exchange: earlier groups' post-exchange
# work overlaps the remaining phase-1 iterations (~1 fixup+stats per
# iteration keeps DVE/Act under the PE pace)
GRPS = ((0, 5), (5, 9), (9, 14), (14, 16))

SX = 16.0            # fp8 scale on x
SW = 512.0           # fp8 scale on projection weights
PSCALE = 1.0 / (SX * SW)

# weight tile section ids (per 128-channel tile): 7 fp8 sections of NDT*128
S_IHH, S_IHL, S_GH, S_GL, S_DEC, S_ZH, S_ZL = range(7)
NSEC = 7


def _build_program(has_out_bias: bool, has_ln_b: bool, sim_no_cc: bool = False,
                   repeat: int = 1, _stage: str = "full"):
    nc = bacc.Bacc("TRN2", target_bir_lowering=False, debug=False,
                   enable_asserts=True, num_devices=NCORES)

    # ---------------- I/O ----------------
    xh_d = nc.dram_tensor("xh", [D, TCX], F8, kind="ExternalInput").ap()
    xl_d = nc.dram_tensor("xl", [D, TCX], F8, kind="ExternalInput").ap()
    w7 = nc.dram_tensor("w7", [D, NCT * NSEC * 128], F8,
                        kind="ExternalInput").ap()
    wcomb = nc.dram_tensor("wcomb", [H, D], BF16, kind="ExternalInput").ap()
    # per-channel vectors in [partition, ch_tile] layout
    def cvec(name, n=NCT):
        return nc.dram_tensor(name, [128, n], F32, kind="ExternalInput").ap()
    bias_ih = cvec("bias_ih")
    bias_halo = cvec("bias_halo")
    bias_gneg = cvec("bias_gneg")
    bias_dec = cvec("bias_dec")
    bias_z = cvec("bias_z")
    conv_b_d = cvec("conv_b")
    ln_g_d = cvec("ln_g")
    convw_d = cvec("convw", NCT * 4)
    onehot_d = cvec("onehot", NCORES)
    if has_out_bias:
        out_bias_d = nc.dram_tensor("out_bias", [128, D], BF16,
                                    kind="ExternalInput").ap()
    if has_ln_b:
        wcomb_b = nc.dram_tensor("wcomb_b", [H, D], BF16,
                                 kind="ExternalInput").ap()
    out = nc.dram_tensor("out", [TC, D], F32, kind="ExternalOutput").ap()

    with tile.TileContext(nc) as tc, ExitStack() as ctx:
        dram = ctx.enter_context(tc.tile_pool(name="dram", bufs=1, space="DRAM"))
        rstd_d = dram.tile([2, TC], F32)

        # ---------------- constants / small resident ----------------
        consts = ctx.enter_context(tc.tile_pool(name="consts", bufs=1))

        def load_const(name, ap_in, n):
            # Act-queue DMAs: keep the SP queue free for the startup x/w loads
            t = consts.tile([128, n], F32, tag=name)
            nc.scalar.dma_start(t[:], ap_in[:, :])
            return t
        bih_s = load_const("bih", bias_ih, NCT)
        bhalo_s = load_const("bhalo", bias_halo, NCT)
        bgn_s = load_const("bgn", bias_gneg, NCT)
        bdec_s = load_const("bdec", bias_dec, NCT)
        bz_s = load_const("bz", bias_z, NCT)
        cb_s = load_const("cb", conv_b_d, NCT)
        lng_s = load_const("lng", ln_g_d, NCT)
        cw_s = load_const("cw", convw_d, NCT * 4)
        oh_s = load_const("oh", onehot_d, NCORES)
        if has_out_bias:
            obias_s = consts.tile([128, D], BF16, tag="obias")
            nc.sync.dma_start(obias_s[:], out_bias_d[:, :])
        ones_stat = consts.tile([128, 1], BF16)
        nc.vector.memset(ones_stat[:], 1.0)
        ones_bc = consts.tile([1, 128], BF16)
        nc.vector.memset(ones_bc[:], 1.0)
        one_f32 = consts.tile([1, 1], F32)
        nc.vector.memset(one_f32[:], 1.0)

        # ---------------- big resident tiles ----------------
        res = ctx.enter_context(tc.tile_pool(name="res", bufs=1))
        z_s = res.tile([128, NCT * TC], BF16)
        h_s = res.tile([128, NCT * TC], BF16)
        cumA_s = res.tile([128, NCT * TC], BF16)
        rows = ctx.enter_context(tc.tile_pool(name="rows", bufs=1))
        Pbuf = res.tile([128, NCT], F32)
        Lbuf = res.tile([128, NCT], F32)
        h_in = res.tile([128, NCT], F32)
        # gathered (P,L) pairs, layout [p, ct, core j, {P,L}]
        PL = res.tile([128, NCT * NCORES * 2], F32)
        Hp = res.tile([128, NCT * NCORES], F32)
        rstd_col = res.tile([128, NDT], F32)
        # second half of the fused output weight, resident for the tt-outer
        # cg=1 pass (loaded during the exchange tail)
        wc1_s = res.tile([128, NCT * 512], BF16)
        mu_sb = rows.tile([128, TC], BF16)
        mu_row = rows.tile([1, TC], F32)
        msq_row = rows.tile([1, TC], F32)
        mu_bf = rows.tile([1, TC], BF16)
        wc0head = rows.tile([128, 2 * 512], BF16)
        mu = mu_row[:, :]
        msq = msq_row[:, :]
        cpack = rows.tile([128, 4 * NDT + 1], F32)
        mu_col = cpack[:, 0:NDT]
        msq_col = cpack[:, NDT:2 * NDT]
        var_col = cpack[:, 2 * NDT:3 * NDT]
        sd_col = cpack[:, 3 * NDT:4 * NDT]
        eps_t = cpack[:, 4 * NDT:4 * NDT + 1]

        def emit_body():
            cc_bufs = []
            for gi, (lo, hi) in enumerate(GRPS):
                n = hi - lo
                cci = dram.tile([1, 2 * n * 128], F32, tag=f"cc{gi}i",
                                name=f"cc{gi}i")
                cco = dram.tile([NCORES, 2 * n * 128], F32,
                                addr_space="Shared", tag=f"cc{gi}o",
                                name=f"cc{gi}o")
                cc_bufs.append((cci, cco))

            def comm_grp(gi):
                """Per-batch AllGather of (P,L) for channel tiles [lo,hi);
                prefix-combine (one masked scan over the (ct, core) sequence)
                and select this core's incoming state into h_in[:, lo:hi]."""
                lo, hi = GRPS[gi]
                cci, cco = cc_bufs[gi]
                n = hi - lo
                # payload layout (ct, p, {P,L}) so each per-core readback is
                # one 3-dim DMA with contiguous innermost pairs
                # the final exchange is latency-critical and runs when the Act
                # queue is idle; earlier exchanges keep Act free for evictions
                last = (gi == len(GRPS) - 1)
                deng = nc.scalar if last else nc.sync
                cciV = cci[0:1, :].rearrange("o (ct p pl) -> p (o ct) pl",
                                             p=128, pl=2)
                deng.dma_start(cciV[:, :, 0], Pbuf[:, lo:hi])
                deng.dma_start(cciV[:, :, 1], Lbuf[:, lo:hi])
                if sim_no_cc:
                    # TimelineSim can't model collectives; local DMA stand-in.
                    deng.dma_start(cco[0:1, :], cci[:, :])
                else:
                    nc.gpsimd.collective_compute(
                        "AllGather", OP.bypass,
                        replica_groups=[list(range(NCORES))],
                        ins=[cci.opt()], outs=[cco.opt()])
                PLV = PL[:].rearrange("p (ct j pl) -> p ct j pl",
                                      j=NCORES, pl=2)
                for j in range(NCORES):
                    eng = deng
                    eng.dma_start(
                        PLV[:, lo:hi, j, :],
                        cco[j:j + 1, :].rearrange("o (ct p pl) -> p (o ct) pl",
                                                  p=128, pl=2))
                # zero the P entries at each batch-start core: the combined
                # scan then resets there, so one linear scan over the whole
                # (ct, core) sequence computes every prefix state
                for j in range(0, NCORES, KCHUNKS):
                    nc.vector.memset(PLV[:, lo:hi, j, 0], 0.0)
                HpV = Hp[:].rearrange("p (ct j) -> p ct j", j=NCORES)
                PLf = (PL[:, lo * 2 * NCORES:hi * 2 * NCORES]
                       .rearrange("p (k pl) -> p k pl", pl=2))
                nc.vector.tensor_tensor_scan(
                    Hp[:, lo * NCORES:hi * NCORES], PLf[:, :, 0],
                    PLf[:, :, 1], 0.0, OP.mult, OP.add)
                nc.vector.memset(h_in[:, lo:hi], 0.0)
                for j in range(NCORES):
                    nc.vector.scalar_tensor_tensor(
                        h_in[:, lo:hi], HpV[:, lo:hi, j], oh_s[:, j:j + 1],
                        h_in[:, lo:hi], OP.mult, OP.add)

            # schedule each group's fixup+stats into the iterations between
            # its exchange and the next one (last group lands in the tail);
            # keep the final iteration free so its own chain finishes early
            p3_sched = {}
            for gi, (lo, hi) in enumerate(GRPS[:-1]):
                slot_lo, slot_hi = GRPS[gi][1], GRPS[gi + 1][1]
                nslots = slot_hi - slot_lo
                for i, ct in enumerate(range(lo, hi)):
                    slot = slot_lo + min(i * nslots // (hi - lo), nslots - 1)
                    p3_sched.setdefault(slot, []).append(ct)

            with tc.tile_pool(name="p3ps", bufs=1, space="PSUM") as spool, \
                 tc.tile_pool(name="p3b", bufs=2) as p3pool:
                # one psum bank holds all four stat rows: sum(h) halves on
                # partitions 0/64, sum(h^2) halves on partitions 32/96
                psum_st = spool.tile([128, 512], F32)

                def fixup_ct(ct):
                    """h = h0 + cumA * h_in for one channel tile."""
                    sl = slice(ct * TC, (ct + 1) * TC)
                    nc.vector.scalar_tensor_tensor(
                        h_s[:, sl], cumA_s[:, sl], h_in[:, ct:ct + 1],
                        h_s[:, sl], OP.mult, OP.add)

                def stats_ct(ct):
                    """accumulate sum(h), sum(h^2) over channels via PE."""
                    sl = slice(ct * TC, (ct + 1) * TC)
                    h2 = p3pool.tile([128, TC], BF16, tag="h2")
                    if ct % 2 == 0:
                        nc.vector.tensor_tensor(h2[:], h_s[:, sl], h_s[:, sl],
                                                OP.mult)
                    else:
                        nc.scalar.activation(h2[:], h_s[:, sl], AF.Square)
                    for half in range(2):
                        nc.tensor.matmul(
                            psum_st[64 * half:64 * half + 1, 0:512],
                            ones_stat[:, 0:1],
                            h_s[:, ct * TC + half * 512:
                                ct * TC + (half + 1) * 512],
                            start=(ct == 0), stop=(ct == NCT - 1),
                            tile_position=(0, 64 * half))
                    for half in range(2):
                        nc.tensor.matmul(
                            psum_st[64 * half + 32:64 * half + 33, 0:512],
                            ones_stat[:, 0:1],
                            h2[:, half * 512:(half + 1) * 512],
                            start=(ct == 0), stop=(ct == NCT - 1),
                            tile_position=(0, 64 * half + 32))

                # ============ phase 1: matmuls + gates + local scans ==========
                with tc.tile_pool(name="p1xt", bufs=1) as xtpool, \
                     tc.tile_pool(name="p1w", bufs=2) as wpool, \
                     tc.tile_pool(name="p1ps", bufs=3, space="PSUM") as ppool, \
                     tc.tile_pool(name="p1hs", bufs=1, space="PSUM") as hpool, \
                     tc.tile_pool(name="p1f", bufs=2) as fpool, \
                     tc.tile_pool(name="p1b", bufs=2) as bpool:
                    xh_s = xtpool.tile([128, NDT * TCX], F8)
                    xl_s = xtpool.tile([128, NDT * TCX], F8)

                    def load_w(ct, split=False):
                        # SBUF layout [p, dt, s, c]: per-dt blocks of 7
                        # contiguous 128-col sections (3-dim DMA both sides)
                        w_t = wpool.tile([128, NSEC * NDT * 128], F8, tag="w_t")
                        dst = w_t[:].rearrange("p (dt sc) -> p dt sc",
                                               sc=NSEC * 128)
                        src = (w7[:, ct * NSEC * 128:(ct + 1) * NSEC * 128]
                               .rearrange("(dt p) sc -> p dt sc", p=128))
                        if split:
                            # ih sections first so m=0 can start sooner
                            nc.sync.dma_start(dst[:, :, 0:256], src[:, :, 0:256])
                            nc.sync.dma_start(dst[:, :, 256:NSEC * 128],
                                              src[:, :, 256:NSEC * 128])
                        else:
                            nc.sync.dma_start(dst, src)
                        return w_t

                    # startup order (DMA transfers serialize): ih weight
                    # sections, then x hi/lo in one transfer each, then the
                    # rest of the ct=0 weights
                    w_next = wpool.tile([128, NSEC * NDT * 128], F8, tag="w_t",
                                        name="w_t0")
                    dst0 = w_next[:].rearrange("p (dt sc) -> p dt sc",
                                               sc=NSEC * 128)
                    src0 = w7[:, 0:NSEC * 128].rearrange(
                        "(dt p) sc -> p dt sc", p=128)
                    nc.sync.dma_start(dst0[:, :, 0:256], src0[:, :, 0:256])
                    nc.sync.dma_start(
                        xh_s[:].rearrange("p (dt t) -> p dt t", t=TCX),
                        xh_d[:, :].rearrange("(dt p) t -> p dt t", p=128))
                    nc.sync.dma_start(
                        xl_s[:].rearrange("p (dt t) -> p dt t", t=TCX),
                        xl_d[:, :].rearrange("(dt p) t -> p dt t", p=128))
                    nc.sync.dma_start(dst0[:, :, 256:NSEC * 128],
                                      src0[:, :, 256:NSEC * 128])
                    xhv = xh_s[:].rearrange("p (dt t) -> p dt t", t=TCX)
                    xlv = xl_s[:].rearrange("p (dt t) -> p dt t", t=TCX)

                    for ct in range(NCT):
                        w_t = w_next
                        if ct + 1 < NCT:
                            w_next = load_w(ct + 1)
                        wv = w_t[:].rearrange("p (dt s c) -> p s dt c",
                                              s=NSEC, c=128)

                        def mmgroup(terms, lo, n, ps_ap):
                            """terms: list of (section, x-view); all accumulate
                            into ps_ap as one PSUM group (shared scale)."""
                            ni = len(terms) * NDP
                            i = 0
                            for (s, xv) in terms:
                                for k in range(NDP):
                                    nc.tensor.matmul(
                                        ps_ap,
                                        wv[:, s, 2 * k:2 * k + 2, :],
                                        xv[:, 2 * k:2 * k + 2, lo:lo + n],
                                        start=(i == 0), stop=(i == ni - 1),
                                        perf_mode=DR)
                                    i += 1

                        T_IH = [(S_IHH, xhv), (S_IHL, xhv), (S_IHH, xlv)]
                        T_G = [(S_GH, xhv), (S_GL, xhv)]
                        T_D = [(S_DEC, xhv)]
                        T_Z = [(S_ZH, xhv), (S_ZL, xhv), (S_ZH, xlv)]

                        # m=0: ih over 1027 halo columns -> bf16 ih_pre
                        ih_pre = fpool.tile([128, TCX], BF16, tag="ihpre")
                        ps_halo = hpool.tile([128, HALO], F32, tag="pshalo")
                        mmgroup(T_IH, 0, HALO, ps_halo[:, :])
                        nc.scalar.activation(ih_pre[:, 0:HALO], ps_halo[:, :],
                                             AF.Identity,
                                             bias=bhalo_s[:, ct:ct + 1],
                                             scale=PSCALE)
                        ps = ppool.tile([128, TC], F32, tag="ps")
                        for half in range(2):
                            mmgroup(T_IH, HALO + half * 512, 512,
                                    ps[:, half * 512:(half + 1) * 512])
                        nc.scalar.activation(ih_pre[:, HALO:HALO + TC], ps[:],
                                             AF.Identity,
                                             bias=bih_s[:, ct:ct + 1],
                                             scale=PSCALE)
                        # m=1: gp = sigmoid(-(x@Wg + bg)) = 1-gate
                        gp = fpool.tile([128, TC], BF16, tag="gp")
                        ps = ppool.tile([128, TC], F32, tag="ps")
                        for half in range(2):
                            mmgroup(T_G, HALO + half * 512, 512,
                                    ps[:, half * 512:(half + 1) * 512])
                        nc.scalar.activation(gp[:], ps[:], AF.Sigmoid,
                                             bias=bgn_s[:, ct:ct + 1],
                                             scale=-PSCALE)
                        # m=2: a = sigmoid(x@Wd + bd)
                        a_t = fpool.tile([128, TC], BF16, tag="a")
                        ps = ppool.tile([128, TC], F32, tag="ps")
                        for half in range(2):
                            mmgroup(T_D, HALO + half * 512, 512,
                                    ps[:, half * 512:(half + 1) * 512])
                        nc.scalar.activation(a_t[:], ps[:], AF.Sigmoid,
                                             bias=bdec_s[:, ct:ct + 1],
                                             scale=PSCALE)
                        # m=3: z = silu(v) = v * sigmoid(v), avoiding the silu
                        # activation table (stays in the sigmoid table set)
                        ps = ppool.tile([128, TC], F32, tag="ps")
                        for half in range(2):
                            mmgroup(T_Z, HALO + half * 512, 512,
                                    ps[:, half * 512:(half + 1) * 512])
                        zv = fpool.tile([128, TC], BF16, tag="zv")
                        nc.scalar.activation(zv[:], ps[:], AF.Identity,
                                             bias=bz_s[:, ct:ct + 1],
                                             scale=PSCALE)
                        zsg = fpool.tile([128, TC], BF16, tag="zsg")
                        nc.scalar.activation(zsg[:], ps[:], AF.Sigmoid,
                                             bias=bz_s[:, ct:ct + 1],
                                             scale=PSCALE)
                        nc.gpsimd.tensor_tensor(z_s[:, ct * TC:(ct + 1) * TC],
                                                zv[:], zsg[:], OP.mult)

                        # causal depthwise conv (4 taps, halo in ih_pre)
                        ihc = fpool.tile([128, TC], BF16, tag="ihc")
                        nc.scalar.activation(ihc[:], ih_pre[:, 3:3 + TC],
                                             AF.Identity,
                                             bias=cb_s[:, ct:ct + 1],
                                             scale=cw_s[:, ct * 4 + 3: ct * 4 + 4])
                        for j in (2, 1, 0):
                            nc.vector.scalar_tensor_tensor(
                                ihc[:], ih_pre[:, j:j + TC],
                                cw_s[:, ct * 4 + j: ct * 4 + j + 1],
                                ihc[:], OP.mult, OP.add)

                        # alpha = gp*a ; gate = 1-gp ; beta = gate*ihc
                        alpha_t = bpool.tile([128, TC], BF16, tag="alpha")
                        nc.gpsimd.tensor_tensor(alpha_t[:], gp[:], a_t[:],
                                                OP.mult)
                        gate_t = bpool.tile([128, TC], BF16, tag="gate")
                        nc.scalar.activation(gate_t[:], gp[:], AF.Copy,
                                             bias=1.0, scale=-1.0)
                        beta_t = bpool.tile([128, TC], BF16, tag="beta")
                        nc.gpsimd.tensor_tensor(beta_t[:], gate_t[:], ihc[:],
                                                OP.mult)

                        # local scan (init 0) -> h_s ; cumprod(alpha) -> cumA_s
                        hsl = h_s[:, ct * TC:(ct + 1) * TC]
                        nc.vector.tensor_tensor_scan(hsl, alpha_t[:],
                                                     beta_t[:], 0.0,
                                                     OP.mult, OP.add)
                        casl = cumA_s[:, ct * TC:(ct + 1) * TC]
                        nc.vector.tensor_tensor_scan(casl, alpha_t[:],
                                                     alpha_t[:], 1.0,
                                                     OP.mult, OP.bypass)
                        # last-column extracts ride the Pool queue (it has
                        # slack) so the exchange writes never wait behind the
                        # DVE backlog
                        nc.gpsimd.tensor_copy(Lbuf[:, ct:ct + 1],
                                              hsl[:, TC - 1:TC])
                        nc.gpsimd.tensor_copy(Pbuf[:, ct:ct + 1],
                                              casl[:, TC - 1:TC])

                        if _stage != "p1":
                            for gi, (glo, ghi) in enumerate(GRPS[:-1]):
                                if ct == ghi - 1:
                                    comm_grp(gi)
                            for c3 in p3_sched.get(ct, []):
                                fixup_ct(c3)
                                stats_ct(c3)

                if _stage == "p1":
                    with tc.tile_pool(name="dbg", bufs=1) as dbgp:
                        dbg = dbgp.tile([128, TC], F32)
                        nc.scalar.activation(dbg[:], h_s[:, 0:TC], AF.Copy)
                        nc.sync.dma_start(out[0:128, 0:TC], dbg[:])
                    return

                # ============ last boundary exchange + fixups =================
                comm_grp(len(GRPS) - 1)
                # load the cg=1 output-weight half on-chip during the tail so
                # the final output pass never touches DRAM (after the exchange
                # DMAs -- transfers serialize on the DMA engines)
                wc1v = wc1_s[:].rearrange("p (ct c) -> p ct c", c=512)
                nc.sync.dma_start(
                    wc1v[:, :, :],
                    wcomb[:, 512:1024].rearrange("(ct p) c -> p ct c", p=128))
                nc.sync.dma_start(wc0head[:].rearrange("p (ct c) -> p ct c",
                                                       c=512),
                                  wcomb[0:256, 0:512].rearrange(
                                      "(ct p) c -> p ct c", p=128))
                for ct in range(GRPS[-1][0], NCT):
                    fixup_ct(ct)
                    stats_ct(ct)

                # ============ LN parameters ===================================
                # bf16 mu first: it alone gates the apply phase (broadcast);
                # the f32 rows only feed the rstd chain for the evictions
                for half in range(2):
                    hs = slice(half * 512, (half + 1) * 512)
                    nc.scalar.activation(mu_bf[0:1, hs],
                                         psum_st[64 * half:64 * half + 1, :],
                                         AF.Copy, scale=1.0 / H)
                # broadcast immediately (nested psum pool: the free p1 banks)
                # so the apply phase is not gated on the slower f32/rstd chain
                with tc.tile_pool(name="p4ps", bufs=1, space="PSUM") as bpool4:
                    mu_b = bpool4.tile([128, TC], F32)
                    for half in range(2):
                        nc.tensor.matmul(
                            mu_b[:, half * 512:(half + 1) * 512],
                            ones_bc[0:1, :],
                            mu_bf[0:1, half * 512:(half + 1) * 512])
                    nc.scalar.activation(mu_sb[:, 0:512], mu_b[:, 0:512],
                                         AF.Copy)
                    nc.vector.tensor_copy(mu_sb[:, 512:1024],
                                          mu_b[:, 512:1024])
                for half in range(2):
                    hs = slice(half * 512, (half + 1) * 512)
                    nc.scalar.activation(mu[:, hs],
                                         psum_st[64 * half:64 * half + 1, :],
                                         AF.Copy, scale=1.0 / H)
                    nc.vector.tensor_scalar(
                        msq[:, hs],
                        psum_st[64 * half + 32:64 * half + 33, :],
                        1.0 / H, None, OP.mult)

            if _stage == "p3":
                with tc.tile_pool(name="dbg", bufs=1) as dbgp:
                    dbg = dbgp.tile([128, TC], F32)
                    nc.scalar.activation(dbg[:], h_s[:, 0:TC], AF.Copy)
                    nc.sync.dma_start(out[0:128, 0:TC], dbg[:])
                    nc.sync.dma_start(out[128:129, 0:TC], mu)
                return

            # transpose the f32 stat rows into [tt-column] layout on the
            # PE (replaces a DRAM round-trip); only gates the evictions
            with tc.tile_pool(name="p4tp", bufs=1, space="PSUM") as tpool4:
                tps = tpool4.tile([128, 2 * NDT], F32)
                for s, row in ((0, mu), (1, msq)):
                    for tt in range(NDT):
                        nc.tensor.matmul(
                            tps[:, s * NDT + tt:s * NDT + tt + 1],
                            row[0:1, tt * 128:(tt + 1) * 128],
                            one_f32[0:1, 0:1], is_transpose=True)
                nc.scalar.activation(cpack[:, 0:2 * NDT], tps[:], AF.Copy)
            nc.vector.memset(eps_t, LN_EPS)
            nc.vector.scalar_tensor_tensor(var_col, mu_col, -1.0, mu_col,
                                           OP.mult, OP.mult)
            nc.vector.tensor_tensor(var_col, msq_col, var_col, OP.add)
            nc.scalar.activation(sd_col, var_col, AF.Sqrt, bias=eps_t)
            nc.vector.reciprocal(rstd_col[:], sd_col)

            if has_ln_b:
                # extra term z @ (diag(ln_b) @ wcomb), staged via DRAM while
                # z is still pre-apply (rarely used: ln_b is usually zero)
                y2_d = dram.tile([TC, D], F32, tag="y2d")
                with tc.tile_pool(name="plnw", bufs=4) as lwpool, \
                     tc.tile_pool(name="plnp", bufs=4, space="PSUM") as lppool, \
                     tc.tile_pool(name="plno", bufs=4) as lopool:
                    for tt in range(NDT):
                        for cg in range(2):
                            y2ps = lppool.tile([128, 512], F32, tag="y2ps")
                            for ct in range(NCT):
                                wbt = lwpool.tile([128, 512], BF16, tag="wbt")
                                nc.sync.dma_start(
                                    wbt[:],
                                    wcomb_b[ct * 128:(ct + 1) * 128,
                                            cg * 512:(cg + 1) * 512])
                                nc.tensor.matmul(
                                    y2ps[:],
                                    z_s[:, ct * TC + tt * 128:
                                        ct * TC + (tt + 1) * 128],
                                    wbt[:],
                                    start=(ct == 0), stop=(ct == NCT - 1))
                            y2sb = lopool.tile([128, 512], F32, tag="y2sb")
                            nc.scalar.activation(y2sb[:], y2ps[:], AF.Copy)
                            nc.sync.dma_start(
                                y2_d[tt * 128:(tt + 1) * 128,
                                     cg * 512:(cg + 1) * 512], y2sb[:])

            # ===== phase 4+5 interleaved: LN apply feeding output matmuls =====
            # cg=0 accumulates per-ct as apply results land; cg=1 runs
            # tt-outer so each output block evicts as soon as it completes.
            with tc.tile_pool(name="p45f", bufs=3) as p4pool, \
                 tc.tile_pool(name="p45w", bufs=3) as wspool, \
                 tc.tile_pool(name="p5ps", bufs=8, space="PSUM") as ypool, \
                 tc.tile_pool(name="p5o", bufs=4) as opool:

                def wslice(ct):
                    w_sl = wspool.tile([128, 512], BF16, tag="wsl")
                    nc.sync.dma_start(
                        w_sl[:], wcomb[ct * 128:(ct + 1) * 128, 0:512])
                    return w_sl

                def evict_tt(tt, cg, y_ps):
                    out_sb = opool.tile([128, 512], F32, tag="osb")
                    if tt % 2 == 0:
                        nc.scalar.activation(out_sb[:], y_ps[:], AF.Copy,
                                             scale=rstd_col[:, tt:tt + 1])
                    else:
                        nc.vector.tensor_scalar(out_sb[:], y_ps[:],
                                                rstd_col[:, tt:tt + 1], None,
                                                OP.mult)
                    if has_ln_b:
                        y2sb = opool.tile([128, 512], F32, tag="y2r")
                        nc.sync.dma_start(
                            y2sb[:], y2_d[tt * 128:(tt + 1) * 128,
                                          cg * 512:(cg + 1) * 512])
                        nc.vector.tensor_tensor(out_sb[:], out_sb[:],
                                                y2sb[:], OP.add)
                    if has_out_bias:
                        nc.vector.tensor_tensor(
                            out_sb[:], out_sb[:],
                            obias_s[:, cg * 512:(cg + 1) * 512], OP.add)
                    oeng = nc.sync if tt % 2 == 0 else nc.scalar
                    oeng.dma_start(
                        out[tt * 128:(tt + 1) * 128, cg * 512:(cg + 1) * 512],
                        out_sb[:])

                ytiles = []
                for _tt in range(NDT):
                    ybank = ypool.tile([128, 512], F32, tag="y")
                    ytiles.append(ybank)
                for ct in range(NCT):
                    sl = slice(ct * TC, (ct + 1) * TC)
                    w_sl = wc0head[:, ct * 512:(ct + 1) * 512] if ct < 2 \
                        else wslice(ct)[:]
                    t1 = p4pool.tile([128, TC], BF16, tag="t1")
                    nc.vector.tensor_tensor(t1[:], h_s[:, sl], mu_sb[:],
                                            OP.subtract)
                    nc.vector.scalar_tensor_tensor(z_s[:, sl], t1[:],
                                                   lng_s[:, ct:ct + 1],
                                                   z_s[:, sl],
                                                   OP.mult, OP.mult)
                    for tt in range(NDT):
                        nc.tensor.matmul(
                            ytiles[tt][:],
                            z_s[:, ct * TC + tt * 128: ct * TC + (tt + 1) * 128],
                            w_sl,
                            start=(ct == 0), stop=(ct == NCT - 1))
                for tt in range(NDT):
                    evict_tt(tt, 0, ytiles[tt])

                # cg=1: tt-outer; each block evicts right after its last ct
                for tt in range(NDT):
                    y_ps = ypool.tile([128, 512], F32, tag="y")
                    for ct in range(NCT):
                        nc.tensor.matmul(
                            y_ps[:],
                            z_s[:, ct * TC + tt * 128: ct * TC + (tt + 1) * 128],
                            wc1v[:, ct, :],
                            start=(ct == 0), stop=(ct == NCT - 1))
                    evict_tt(tt, 1, y_ps)

        for _rep in range(repeat):
            emit_body()

    nc.compile()
    return nc


@functools.lru_cache(maxsize=4)
def _program(flags):
    return _build_program(*flags)


def _v2m(v):
    return np.ascontiguousarray(np.asarray(v, np.float32).reshape(NCT, 128).T)


F8NP = ml_dtypes.float8_e4m3


def _split8(w, s):
    """split fp32 array into (hi, lo) fp8 parts sharing scale s."""
    hi = (w * s).astype(F8NP)
    lo = ((w - hi.astype(np.float32) / s) * s).astype(F8NP)
    return hi, lo


def kernel(x, W_ih_w, W_ih_b, W_gate_w, W_gate_b, W_decay_w, W_decay_b,
           W_z_w, W_z_b, conv_w, conv_b, ln_g, ln_b, W_out_w, W_out_b,
           out_w, out_b):
    f32 = np.float32
    bf = ml_dtypes.bfloat16
    x = np.asarray(x, f32)

    out_bias_eff = (np.asarray(W_out_b, f32) @ np.asarray(out_w, f32)
                    + np.asarray(out_b, f32))
    has_ob = bool(np.any(out_bias_eff != 0.0))
    has_lb = bool(np.any(np.asarray(ln_b) != 0.0))
    nc = _program((has_ob, has_lb))

    # fp8 weight sections, packed per channel-tile: [D, NCT*7*128]
    ih_h, ih_l = _split8(np.asarray(W_ih_w, f32), SW)
    g_h, g_l = _split8(np.asarray(W_gate_w, f32), SW)
    d_q = (np.asarray(W_decay_w, f32) * SW).astype(F8NP)
    z_h, z_l = _split8(np.asarray(W_z_w, f32), SW)
    sects = np.stack([ih_h, ih_l, g_h, g_l, d_q, z_h, z_l], axis=1)  # [D,7,H]
    w7 = np.ascontiguousarray(
        sects.reshape(D, NSEC, NCT, 128).transpose(0, 2, 1, 3)
        .reshape(D, NCT * NSEC * 128))

    wcomb = (np.asarray(W_out_w, f32) @ np.asarray(out_w, f32)).astype(bf)
    convw_m = np.ascontiguousarray(
        np.asarray(conv_w, f32).reshape(NCT, 128, 4).transpose(1, 0, 2)
        .reshape(128, NCT * 4))
    bias_ih_m = _v2m(W_ih_b)
    shared = dict(
        w7=w7, wcomb=wcomb, bias_ih=bias_ih_m,
        bias_gneg=_v2m(-np.asarray(W_gate_b, f32)),
        bias_dec=_v2m(W_decay_b), bias_z=_v2m(W_z_b), conv_b=_v2m(conv_b),
        ln_g=_v2m(ln_g), convw=convw_m)
    if has_ob:
        shared["out_bias"] = np.ascontiguousarray(
            np.tile(out_bias_eff[None, :], (128, 1)).astype(bf))
    if has_lb:
        shared["wcomb_b"] = np.ascontiguousarray(
            (np.asarray(ln_b, f32)[:, None]
             * (np.asarray(W_out_w, f32) @ np.asarray(out_w, f32))).astype(bf))

    zero_halo = np.zeros((128, NCT), f32)
    in_maps = []
    for c in range(NCORES):
        b, k = divmod(c, KCHUNKS)
        t0 = k * TC
        pad = np.zeros((1, D), f32)
        if k == 0:
            xc = np.vstack([np.zeros((HALO, D), f32), x[b, :TC], pad])
        else:
            xc = np.vstack([x[b, t0 - HALO: t0 + TC], pad])
        xb = np.ascontiguousarray(xc.T).astype(bf).astype(f32)
        xh8, xl8 = _split8(xb, SX)
        oh = np.zeros(NCORES, f32)
        if k > 0:
            oh[b * KCHUNKS + k - 1] = 1.0
        in_maps.append({
            **shared,
            "xh": np.ascontiguousarray(xh8),
            "xl": np.ascontiguousarray(xl8),
            "bias_halo": bias_ih_m if k > 0 else zero_halo,
            "onehot": np.ascontiguousarray(np.tile(oh[None, :], (128, 1))),
        })

    res = run_bass_kernel_spmd(nc, in_maps, core_ids=list(range(NCORES)))

    outf = np.empty((B, T, D), f32)
    for c in range(NCORES):
        b, k = divmod(c, KCHUNKS)
        outf[b, k * TC:(k + 1) * TC, :] = res.results[c]["out"]
    return outf
